# revision 1
# baseline (speedup 1.0000x reference)
"""Trainium2 Bass kernel for nn_DHDN_Dynamic (hypergraph GNN + attention + VAE).

Self-contained: takes FULL inputs as numpy arrays, shards batch over 8
NeuronCores (pure data parallel), runs one Bass/Tile kernel per core,
gathers the full output.
"""
import sys
sys.path.insert(0, '/opt/trn_rl_repo')
import numpy as np

import concourse.bass as bass
from concourse import bacc
import concourse.mybir as mybir
from concourse.tile import TileContext
from concourse.bass_utils import run_bass_kernel_spmd
from concourse.masks import make_identity

F32 = mybir.dt.float32
U32 = mybir.dt.uint32
AF = mybir.ActivationFunctionType
OP = mybir.AluOpType

B, J, H, MAXK, PLAT = 4096, 19, 256, 8, 64
NHEAD, DH = 4, 64
NCORES = 8
BC = B // NCORES          # graphs per core
GPT = 6                   # graphs per (block-diag) tile
RF = GPT * J              # 114 rows per full tile
NEG = -1.0e9

_CACHE = {}


def _tiles(bc):
    """List of (tile_idx, g0, G) covering bc graphs in 6-graph tiles."""
    out = []
    g0 = 0
    t = 0
    while g0 < bc:
        G = min(GPT, bc - g0)
        out.append((t, g0, G))
        g0 += G
        t += 1
    return out


def _chunks(bc):
    """Two chunks of graphs: [(g0, g1, tiles)]"""
    tl = _tiles(bc)
    half = (len(tl) + 1) // 2
    c0 = tl[:half]
    c1 = tl[half:]
    out = []
    for ts_ in (c0, c1):
        if not ts_:
            continue
        g0 = ts_[0][1]
        g1 = ts_[-1][1] + ts_[-1][2]
        out.append((g0, g1, ts_))
    return out


def build_nc(bc=BC, stages=4, sub=9):
    """Build the Bass IR for one core processing bc graphs."""
    nc = bacc.Bacc("TRN2", target_bir_lowering=False)
    R = bc * J  # total rows (tokens)

    # ---------------- DRAM I/O ----------------
    d_pts = nc.dram_tensor("pts", [R, 2], F32, kind="ExternalInput")
    d_feat = nc.dram_tensor("feat", [R, H], F32, kind="ExternalInput")
    d_kvrem = nc.dram_tensor("kvrem", [R, 8], F32, kind="ExternalInput")
    d_rinv = nc.dram_tensor("rinv", [R, 1], F32, kind="ExternalInput")
    d_bo = nc.dram_tensor("bo", [RF, RF], F32, kind="ExternalInput")
    d_bo4 = nc.dram_tensor("bo4", [RF, 512], F32, kind="ExternalInput")
    d_epsT = nc.dram_tensor("epsT", [PLAT, bc + 8], F32, kind="ExternalInput")
    d_wc0 = nc.dram_tensor("wc0", [3, H], F32, kind="ExternalInput")
    d_wga = nc.dram_tensor("wga", [128, 2, H], F32, kind="ExternalInput")   # layers 1,2 half0
    d_wgb = nc.dram_tensor("wgb", [128, 2, H], F32, kind="ExternalInput")   # layers 1,2 half1
    d_cgn = nc.dram_tensor("cgn", [2, H], F32, kind="ExternalInput")
    d_bg3 = nc.dram_tensor("bg3", [3, H], F32, kind="ExternalInput")        # b_gnn rows
    d_wqa = nc.dram_tensor("wqa", [128, 3 * H], F32, kind="ExternalInput")
    d_wqb = nc.dram_tensor("wqb", [128, 3 * H], F32, kind="ExternalInput")
    d_cq = nc.dram_tensor("cq", [1, 3 * H], F32, kind="ExternalInput")
    d_wao4 = nc.dram_tensor("wao4", [64, 4, H], F32, kind="ExternalInput")
    d_cao = nc.dram_tensor("cao", [1, H], F32, kind="ExternalInput")
    d_we1 = nc.dram_tensor("we1", [128, 38, H], F32, kind="ExternalInput")
    d_be1 = nc.dram_tensor("be1", [128, 2], F32, kind="ExternalInput")
    d_we2 = nc.dram_tensor("we2", [128, 2, 2, 64], F32, kind="ExternalInput")
    d_be2 = nc.dram_tensor("be2", [64, 2], F32, kind="ExternalInput")
    d_wdec = nc.dram_tensor("wdec", [PLAT, 38, 128], F32, kind="ExternalInput")
    d_bdec = nc.dram_tensor("bdec", [128, 38], F32, kind="ExternalInput")
    d_wr1 = nc.dram_tensor("wr1", [128, 38, H], F32, kind="ExternalInput")
    d_br1 = nc.dram_tensor("br1", [128, 2], F32, kind="ExternalInput")
    d_wr2 = nc.dram_tensor("wr2", [128, 2, 38], F32, kind="ExternalInput")
    d_br2 = nc.dram_tensor("br2", [38, 1], F32, kind="ExternalInput")
    d_y = nc.dram_tensor("y", [bc, 38], F32, kind="ExternalOutput")

    chunks = _chunks(bc)
    NV = max(g1 - g0 for g0, g1, _ in chunks)  # VAE free dim (padded)
    NV = max(NV, chunks[0][1] - chunks[0][0])

    with TileContext(nc) as tc:
        with tc.tile_pool(name="cst", bufs=1) as cst, \
             tc.tile_pool(name="wk", bufs=4) as wk, \
             tc.tile_pool(name="hbuf", bufs=1) as hbuf, \
             tc.tile_pool(name="io", bufs=3) as io, \
             tc.tile_pool(name="sbA", bufs=3) as sbA, \
             tc.tile_pool(name="sbB", bufs=2) as sbB, \
             tc.tile_pool(name="sbC", bufs=2) as sbC, \
             tc.tile_pool(name="small", bufs=4) as small, \
             tc.tile_pool(name="pm", bufs=3, space="PSUM") as pm, \
             tc.tile_pool(name="pr", bufs=1, space="PSUM") as pr, \
             tc.tile_pool(name="pt", bufs=2, space="PSUM") as pt:

            # ----- constants -----
            ident = cst.tile([128, 128], F32)
            make_identity(nc, ident[:])
            zcol = cst.tile([128, 1], F32)
            nc.vector.memset(zcol[:], 0.0)
            negt = cst.tile([128, 8], F32)
            nc.vector.memset(negt[:], -2.0e9)
            epsc = cst.tile([128, 1], F32)
            nc.vector.memset(epsc[:], 1.0e-5)
            bo_t = cst.tile([RF, RF], F32)
            nc.sync.dma_start(bo_t[:], d_bo[:])
            bo4_t = cst.tile([RF, 512], F32)
            nc.sync.dma_start(bo4_t[:], d_bo4[:])
            wc0_t = cst.tile([3, H], F32)
            nc.sync.dma_start(wc0_t[:], d_wc0[:])
            wga_t = cst.tile([128, 2, H], F32)
            nc.sync.dma_start(wga_t[:], d_wga[:])
            wgb_t = cst.tile([128, 2, H], F32)
            nc.sync.dma_start(wgb_t[:], d_wgb[:])
            cgnb_t = cst.tile([128, 2, H], F32)
            nc.sync.dma_start(cgnb_t[:], d_cgn[None, :, :].to_broadcast([128, 2, H]))
            bg3_t = cst.tile([3, H], F32)
            nc.sync.dma_start(bg3_t[:], d_bg3[:])
            wqa_t = cst.tile([128, 3 * H], F32)
            nc.sync.dma_start(wqa_t[:], d_wqa[:])
            wqb_t = cst.tile([128, 3 * H], F32)
            nc.sync.dma_start(wqb_t[:], d_wqb[:])
            cqb_t = cst.tile([128, 3 * H], F32)
            nc.sync.dma_start(cqb_t[:], d_cq[None, 0, :].to_broadcast([128, 3 * H]))
            wao4_t = cst.tile([64, 4, H], F32)
            nc.sync.dma_start(wao4_t[:], d_wao4[:])
            caob_t = cst.tile([128, H], F32)
            nc.sync.dma_start(caob_t[:], d_cao[None, 0, :].to_broadcast([128, H]))
            we2_t = cst.tile([128, 2, 2, 64], F32)
            nc.sync.dma_start(we2_t[:], d_we2[:])
            be1_t = cst.tile([128, 2], F32)
            nc.sync.dma_start(be1_t[:], d_be1[:])
            be2_t = cst.tile([64, 2], F32)
            nc.sync.dma_start(be2_t[:], d_be2[:])
            be2h_t = cst.tile([64, 1], F32)
            nc.vector.tensor_scalar_mul(be2h_t[:], be2_t[:, 1:2], 0.5)
            bdec_t = cst.tile([128, 38], F32)
            nc.sync.dma_start(bdec_t[:], d_bdec[:])
            br1_t = cst.tile([128, 2], F32)
            nc.sync.dma_start(br1_t[:], d_br1[:])
            wr2_t = cst.tile([128, 2, 38], F32)
            nc.sync.dma_start(wr2_t[:], d_wr2[:])
            br2_t = cst.tile([38, 1], F32)
            nc.sync.dma_start(br2_t[:], d_br2[:])
            epsT_t = cst.tile([PLAT, bc + 8], F32)
            nc.sync.dma_start(epsT_t[:], d_epsT[:])
            wdec_t = cst.tile([PLAT, 38, 128], F32)
            nc.sync.dma_start(wdec_t[:], d_wdec[:])

            # hfT assembly buffer: [128, half, i(19), NV]
            hfT = hbuf.tile([128, 2, J, NV], F32)

            def process_tile(tinfo, cg0):
                t, g0, G = tinfo
                Rt = G * J
                r0 = g0 * J
                bo = bo_t[:Rt, :Rt]

                # --- input DMAs ---
                ft = io.tile([RF, H], F32, tag="feat_in")
                nc.sync.dma_start(ft[:Rt, :], d_feat[r0:r0 + Rt, :])
                ptst = io.tile([RF, 2], F32, tag="pts_in")
                nc.sync.dma_start(ptst[:Rt, :], d_pts[r0:r0 + Rt, :])
                kvt = io.tile([RF, 8], F32, tag="kv_in")
                nc.sync.dma_start(kvt[:Rt, :], d_kvrem[r0:r0 + Rt, :])
                rit = io.tile([RF, 1], F32, tag="ri_in")
                nc.sync.dma_start(rit[:Rt, :], d_rinv[r0:r0 + Rt, :])

                # --- norms: nf = |F|^2 rows, npts = |p|^2 rows ---
                sqsc = sbB.tile([RF, H], F32, tag="sqscratch")
                nf = small.tile([RF, 1], F32, tag="nf")
                nc.scalar.activation(sqsc[:Rt, :], ft[:Rt, :], AF.Square,
                                     accum_out=nf[:Rt, :])
                np_ = small.tile([RF, 1], F32, tag="npts")
                nc.scalar.activation(sqsc[:Rt, 0:2], ptst[:Rt, :], AF.Square,
                                     accum_out=np_[:Rt, :])

                # --- transposes of F (2 chunks), pts, nf, npts ---
                fT = sbA.tile([128, 2, RF], F32, tag="fT")
                for c in range(2):
                    p = pt.tile([128, RF], F32, tag="ptrans")
                    nc.tensor.transpose(p[:, :Rt], ft[:Rt, c * 128:(c + 1) * 128],
                                        ident[:Rt, :Rt])
                    nc.scalar.copy(fT[:, c, :Rt], p[:, :Rt])
                # ptsT_ext [3, Rt]: rows 0:2 = pts^T, row 2 = ones
                ptsT = sbA.tile([3, RF], F32, tag="ptsT")
                nc.vector.memset(ptsT[0:3, :Rt], 1.0)
                p = pt.tile([128, RF], F32, tag="ptrans")
                nc.tensor.transpose(p[:2, :Rt], ptst[:Rt, :], ident[:Rt, :Rt])
                nc.scalar.copy(ptsT[0:2, :Rt], p[:2, :Rt])
                # norm-row pairs built by transposing [Rt, 2] column tiles:
                # cols (0,1)=(1, -nf/2) (2,3)=(-nf/2, 1) (4,5)=(1, -np/2) (6,7)=(-np/2, 1)
                normin = sbA.tile([RF, 8], F32, tag="normin")
                nc.vector.memset(normin[:Rt, :], 1.0)
                nc.vector.tensor_scalar_mul(normin[:Rt, 1:2], nf[:Rt, :], -0.5)
                nc.vector.tensor_scalar_mul(normin[:Rt, 2:3], nf[:Rt, :], -0.5)
                nc.vector.tensor_scalar_mul(normin[:Rt, 5:6], np_[:Rt, :], -0.5)
                nc.vector.tensor_scalar_mul(normin[:Rt, 6:7], np_[:Rt, :], -0.5)
                nrows = sbA.tile([2, 4, RF], F32, tag="nrows")
                for jj in range(4):
                    p = pt.tile([128, RF], F32, tag="ptrans")
                    nc.tensor.transpose(p[:2, :Rt], normin[:Rt, 2 * jj:2 * jj + 2],
                                        ident[:Rt, :Rt])
                    nc.scalar.copy(nrows[:, jj, :Rt], p[:2, :Rt])

                # --- Gram(feat) - 0.5 nf_i - 0.5 nf_j  (= -0.5 * d2f) ---
                gm = pm.tile([128, 512], F32, tag="pbig")
                nc.tensor.matmul(gm[:Rt, :Rt], fT[:, 0, :Rt], fT[:, 0, :Rt],
                                 start=True, stop=False)
                nc.tensor.matmul(gm[:Rt, :Rt], fT[:, 1, :Rt], fT[:, 1, :Rt],
                                 start=False, stop=False)
                nc.tensor.matmul(gm[:Rt, :Rt], nrows[:, 0, :Rt], nrows[:, 1, :Rt],
                                 start=False, stop=True)
                gmin = sbB.tile([RF, RF], F32, tag="gmin")
                nc.vector.tensor_scalar(gmin[:Rt, :Rt], gm[:Rt, :Rt], 0.0, None, OP.min)
                sf = sbB.tile([RF, RF], F32, tag="sf")
                nc.scalar.activation(sf[:Rt, :Rt], gmin[:Rt, :Rt], AF.Sqrt, scale=-2.0)
                # --- Gram(pts) ---
                gp = pm.tile([128, 512], F32, tag="pbig")
                nc.tensor.matmul(gp[:Rt, :Rt], ptsT[0:2, :Rt], ptsT[0:2, :Rt],
                                 start=True, stop=False)
                nc.tensor.matmul(gp[:Rt, :Rt], nrows[:, 2, :Rt], nrows[:, 3, :Rt],
                                 start=False, stop=True)
                nc.vector.tensor_scalar(gmin[:Rt, :Rt], gp[:Rt, :Rt], 0.0, None, OP.min)
                sp = sbB.tile([RF, RF], F32, tag="sp")
                nc.scalar.activation(sp[:Rt, :Rt], gmin[:Rt, :Rt], AF.Sqrt, scale=-2.0)

                # score = BO - (sf + sp)   (on-block: -(df+dp); off-block <= -1e9)
                nc.vector.tensor_tensor(sf[:Rt, :Rt], sf[:Rt, :Rt], sp[:Rt, :Rt], OP.add)
                score = sbB.tile([RF, RF], F32, tag="score")
                nc.vector.tensor_tensor(score[:Rt, :Rt], bo[:Rt, :Rt], sf[:Rt, :Rt],
                                        OP.subtract)

                # --- top-kv mask S ---
                mx = small.tile([RF, 8], F32, tag="mx")
                nc.vector.max(out=mx[:Rt, :], in_=score[:Rt, :Rt])
                done = small.tile([RF, 8], U32, tag="done")
                nc.vector.tensor_scalar(done[:Rt, :], kvt[:Rt, :], 0.0, None, OP.is_le)
                nc.vector.copy_predicated(mx[:Rt, :], done[:Rt, :], negt[:Rt, :])
                rep = sbB.tile([RF, RF], F32, tag="rep")
                nc.vector.match_replace(out=rep[:Rt, :Rt], in_to_replace=mx[:Rt, :],
                                        in_values=score[:Rt, :Rt], imm_value=NEG)
                S = sbA.tile([RF, RF], F32, tag="S")
                nc.vector.tensor_tensor(S[:Rt, :Rt], score[:Rt, :Rt], rep[:Rt, :Rt],
                                        OP.is_gt)

                # --- A matrix: Af [Rt+1, Rt], rows 0:Rt = (diag(Dinv) Araw)^T, last = 1
                SR = sbB.tile([RF, RF + 1], F32, tag="SR")
                nc.vector.tensor_scalar(SR[:Rt, :Rt], S[:Rt, :Rt], rit[:Rt, :], None,
                                        OP.mult)
                nc.vector.memset(SR[:Rt, Rt:Rt + 1], 1.0)
                araw = pm.tile([128, 512], F32, tag="pbig")
                nc.tensor.matmul(araw[:Rt, :Rt + 1], S[:Rt, :Rt], SR[:Rt, :Rt + 1],
                                 start=True, stop=True)
                dinv = small.tile([RF, 1], F32, tag="dinv")
                nc.vector.reciprocal(dinv[:Rt, :], araw[:Rt, Rt:Rt + 1])
                dz = small.tile([RF, 1], U32, tag="dz")
                nc.vector.tensor_scalar(dz[:Rt, :], araw[:Rt, Rt:Rt + 1], 0.0, None,
                                        OP.is_le)
                nc.vector.copy_predicated(dinv[:Rt, :], dz[:Rt, :], zcol[:Rt, :])
                asc = sbB.tile([RF, RF + 1], F32, tag="asc")
                nc.vector.tensor_scalar(asc[:Rt, :Rt], araw[:Rt, :Rt], dinv[:Rt, :],
                                        None, OP.mult)
                nc.vector.memset(asc[:Rt, Rt:Rt + 1], 1.0)
                pA = pt.tile([128, RF], F32, tag="ptrans")
                nc.tensor.transpose(pA[:Rt + 1, :Rt], asc[:Rt, :Rt + 1],
                                    ident[:Rt, :Rt])
                Af = sbA.tile([RF + 1, RF], F32, tag="Af")
                nc.scalar.copy(Af[:Rt + 1, :Rt], pA[:Rt + 1, :Rt])

                if stages < 2:
                    nc.sync.dma_start(d_y[g0:g0 + G, :],
                                      Af[:G, :38])
                    return
                # ---------------- 3 hconv layers ----------------
                h = None
                for l in range(3):
                    xt = pm.tile([128, 512], F32, tag="pbig")
                    if l == 0:
                        nc.tensor.matmul(xt[:Rt, :H], ptsT[:, :Rt], wc0_t[:],
                                         start=True, stop=True)
                    else:
                        hT = sbC.tile([128, 2, RF], F32, tag="hT")
                        for c in range(2):
                            p2 = pt.tile([128, RF], F32, tag="ptrans")
                            nc.tensor.transpose(p2[:, :Rt],
                                                h[:Rt, c * 128:(c + 1) * 128],
                                                ident[:Rt, :Rt])
                            nc.scalar.copy(hT[:, c, :Rt], p2[:, :Rt])
                        wl = l - 1
                        nc.tensor.matmul(xt[:Rt, :H], hT[:, 0, :Rt],
                                         wga_t[:, wl, :], start=True, stop=False)
                        nc.tensor.matmul(xt[:Rt, :H], hT[:, 1, :Rt],
                                         wgb_t[:, wl, :], start=False, stop=True)
                    xts = sbC.tile([RF + 1, H], F32, tag="xts")
                    if l == 0:
                        nc.vector.tensor_copy(xts[:Rt, :], xt[:Rt, :H])
                    else:
                        nc.vector.tensor_tensor(xts[:Rt, :], xt[:Rt, :H],
                                                cgnb_t[:Rt, l - 1, :], OP.add)
                    nc.sync.dma_start(xts[Rt:Rt + 1, :], d_bg3[l:l + 1, :])
                    agg = pm.tile([128, 512], F32, tag="pbig")
                    nc.tensor.matmul(agg[:Rt, :H], Af[:Rt + 1, :Rt], xts[:Rt + 1, :],
                                     start=True, stop=True)
                    # relu + LN stats
                    hr = sbC.tile([RF, H], F32, tag="hrelu")
                    rsum = small.tile([RF, 1], F32, tag="rsum")
                    nc.scalar.activation(hr[:Rt, :], agg[:Rt, :H], AF.Relu,
                                         accum_out=rsum[:Rt, :])
                    ssq = small.tile([RF, 1], F32, tag="ssq")
                    nc.scalar.activation(sqsc[:Rt, :], hr[:Rt, :], AF.Square,
                                         accum_out=ssq[:Rt, :])
                    mu = small.tile([RF, 1], F32, tag="mu")
                    nc.vector.tensor_scalar_mul(mu[:Rt, :], rsum[:Rt, :], 1.0 / H)
                    var = small.tile([RF, 1], F32, tag="var")
                    nc.vector.tensor_scalar_mul(var[:Rt, :], ssq[:Rt, :], 1.0 / H)
                    mu2 = small.tile([RF, 1], F32, tag="mu2")
                    nc.vector.tensor_tensor(mu2[:Rt, :], mu[:Rt, :], mu[:Rt, :], OP.mult)
                    nc.vector.tensor_tensor(var[:Rt, :], var[:Rt, :], mu2[:Rt, :],
                                            OP.subtract)
                    sg = small.tile([RF, 1], F32, tag="sg")
                    nc.scalar.activation(sg[:Rt, :], var[:Rt, :], AF.Sqrt, bias=epsc[:Rt, :])
                    rs = small.tile([RF, 1], F32, tag="rs")
                    nc.vector.reciprocal(rs[:Rt, :], sg[:Rt, :])
                    h = sbC.tile([RF, H], F32, tag=f"h{l}")
                    nc.vector.tensor_scalar(h[:Rt, :], hr[:Rt, :], mu[:Rt, :],
                                            rs[:Rt, :], OP.subtract, OP.mult)

                if stages < 3:
                    nc.sync.dma_start(d_y[g0:g0 + G, :], h[:G, :38])
                    return
                # ---------------- attention ----------------
                hT = sbC.tile([128, 2, RF], F32, tag="hT")
                for c in range(2):
                    p2 = pt.tile([128, RF], F32, tag="ptrans")
                    nc.tensor.transpose(p2[:, :Rt], h[:Rt, c * 128:(c + 1) * 128],
                                        ident[:Rt, :Rt])
                    nc.scalar.copy(hT[:, c, :Rt], p2[:, :Rt])
                qkvs = sbB.tile([RF, 3 * H], F32, tag="qkvs")
                for nh in range(2):
                    qkv = pm.tile([128, 512], F32, tag="pbig")
                    s0, s1 = nh * 384, (nh + 1) * 384
                    nc.tensor.matmul(qkv[:Rt, :384], hT[:, 0, :Rt],
                                     wqa_t[:, s0:s1], start=True, stop=False)
                    nc.tensor.matmul(qkv[:Rt, :384], hT[:, 1, :Rt],
                                     wqb_t[:, s0:s1], start=False, stop=True)
                    nc.vector.tensor_tensor(qkvs[:Rt, s0:s1], qkv[:Rt, :384],
                                            cqb_t[:Rt, s0:s1], OP.add)
                if sub < 1:
                    nc.sync.dma_start(d_y[g0:g0 + G, :], qkvs[:G, :38])
                    return
                # q,k transposed per head-pair; q scaled by 1/8
                qT = sbA.tile([64, 4, RF], F32, tag="qT")
                kT = sbA.tile([64, 4, 128], F32, tag="kT")
                if Rt < 128:
                    nc.vector.memset(kT[:, :, Rt:128], 0.0)
                for hh in range(4):
                    p2 = pt.tile([128, RF], F32, tag="ptrans")
                    nc.tensor.transpose(p2[:64, :Rt], qkvs[:Rt, hh * 64:(hh + 1) * 64],
                                        ident[:Rt, :Rt])
                    nc.scalar.mul(qT[:, hh, :Rt], p2[:64, :Rt], 0.125)
                    p2 = pt.tile([128, RF], F32, tag="ptrans")
                    nc.tensor.transpose(p2[:64, :Rt],
                                        qkvs[:Rt, H + hh * 64:H + (hh + 1) * 64],
                                        ident[:Rt, :Rt])
                    nc.scalar.copy(kT[:, hh, :Rt], p2[:64, :Rt])
                if sub < 2:
                    nc.sync.dma_start(d_y[g0:g0 + G, :], qT[:G, 0, :38])
                    return
                # scores for 4 heads into one psum [Rt, 4*Rt]
                scs = []
                for hh in range(4):
                    sc = pm.tile([128, 512], F32, tag="pbig", name=f"sc{hh}")
                    nc.tensor.matmul(sc[:Rt, :128], qT[:, hh, :Rt], kT[:, hh, :],
                                     start=True, stop=True)
                    scs.append(sc)
                if sub == 2.01:
                    tmpo = sbB.tile([RF, 4 * RF], F32, tag="tmpo")
                    nc.vector.tensor_copy(tmpo[:Rt, :4 * Rt], sc[:Rt, :4 * Rt])
                    nc.sync.dma_start(d_y[g0:g0 + G, :], tmpo[:G, :38])
                    return
                pexp = sbB.tile([RF, 512], F32, tag="pexp")
                for hh in range(4):
                    nc.scalar.activation(pexp[:Rt, hh * 128:(hh + 1) * 128],
                                         scs[hh][:Rt, :128], AF.Exp)
                # mask off-block + per-head row sums
                if sub == 2.02:
                    nc.sync.dma_start(d_y[g0:g0 + G, :], pexp[:G, :38])
                    return
                sums = small.tile([RF, 4], F32, tag="sums")
                pm4 = sbB.tile([RF, 512], F32, tag="pm4")
                nc.vector.tensor_tensor(
                    pm4[:Rt, :].rearrange("p (h j) -> p h j", h=4),
                    pexp[:Rt, :].rearrange("p (h j) -> p h j", h=4),
                    bo4_t[:Rt, :].rearrange("p (h j) -> p h j", h=4), OP.mult)
                nc.vector.tensor_reduce(
                    sums[:Rt, :],
                    pm4[:Rt, :].rearrange("p (h j) -> p h j", h=4),
                    axis=mybir.AxisListType.X, op=OP.add)
                if sub == 2.03:
                    nc.sync.dma_start(d_y[g0:g0 + G, :], pm4[:G, :38])
                    return
                rsum4 = small.tile([RF, 4], F32, tag="rsum4")
                nc.vector.reciprocal(rsum4[:Rt, :], sums[:Rt, :])
                if sub == 2.04:
                    nc.sync.dma_start(d_y[g0:g0 + G, :], sums[:G, :4])
                    return
                att = sbB.tile([RF, 512], F32, tag="att")
                nc.vector.tensor_tensor(
                    att[:Rt, :].rearrange("p (h j) -> p h j", h=4),
                    pm4[:Rt, :].rearrange("p (h j) -> p h j", h=4),
                    rsum4[:Rt, :, None].to_broadcast((Rt, 4, 128)),
                    OP.mult)
                if sub < 3:
                    nc.sync.dma_start(d_y[g0:g0 + G, :], att[:G, :38])
                    return
                # attT per head, then AV; oT4 [64, 4, RF]
                oT = sbA.tile([64, 4, RF], F32, tag="oT")
                for hh in range(4):
                    pa = pt.tile([128, RF], F32, tag="ptrans")
                    nc.tensor.transpose(pa[:Rt, :Rt],
                                        att[:Rt, hh * 128:hh * 128 + Rt],
                                        ident[:Rt, :Rt])
                    aT = sbB.tile([RF, RF], F32, tag="aT")
                    nc.scalar.copy(aT[:Rt, :Rt], pa[:Rt, :Rt])
                    po = pm.tile([128, 512], F32, tag="pbig", name=f"po{hh}")
                    nc.tensor.matmul(po[:64, :Rt], qkvs[:Rt, 512 + hh * 64:512 + (hh + 1) * 64],
                                     aT[:Rt, :Rt], start=True, stop=True)
                    nc.scalar.copy(oT[:, hh, :Rt], po[:64, :Rt])
                if sub < 4:
                    nc.sync.dma_start(d_y[g0:g0 + G, :], oT[:G, 0, :38])
                    return
                hat = pm.tile([128, 512], F32, tag="pbig")
                for hh in range(4):
                    nc.tensor.matmul(hat[:Rt, :H], oT[:, hh, :Rt], wao4_t[:, hh, :],
                                     start=(hh == 0), stop=(hh == 3))
                hats = sbC.tile([RF, H], F32, tag="hats")
                nc.vector.tensor_tensor(hats[:Rt, :], hat[:Rt, :H],
                                        caob_t[:Rt, :], OP.add)
                # scatter transposed into hfT
                col0 = g0 - cg0
                for c in range(2):
                    p2 = pt.tile([128, RF], F32, tag="ptrans")
                    nc.tensor.transpose(p2[:, :Rt], hats[:Rt, c * 128:(c + 1) * 128],
                                        ident[:Rt, :Rt])
                    dst = hfT[:, c, :, col0:col0 + G].rearrange("p i g -> p g i")
                    nc.vector.tensor_copy(dst, p2[:, :Rt].rearrange(
                        "p (g i) -> p g i", i=J))

            def vae(cg0, cg1):
                GC = cg1 - cg0
                if GC < NV:
                    nc.vector.memset(hfT[:, :, :, GC:NV], 0.0)
                # e1: out e1r [2][128, NV]; stream we1 per-kk
                pse = [pr.tile([128, 512], F32, tag=f"pr1_{m}", name=f"pse{m}")
                       for m in range(2)]
                for kk in range(38):
                    i, half = kk // 2, kk % 2
                    wkt = wk.tile([128, H], F32, tag="wk1")
                    nc.sync.dma_start(wkt[:], d_we1[:, kk, :])
                    for m in range(2):
                        nc.tensor.matmul(pse[m][:, :NV], wkt[:, m * 128:(m + 1) * 128],
                                         hfT[:, half, i, :], start=(kk == 0),
                                         stop=(kk == 37))
                e1r = []
                for m in range(2):
                    r = sbB.tile([128, NV], F32, tag=f"e1r{m}", name=f"e1r{m}")
                    nc.scalar.activation(r[:], pse[m][:, :NV], AF.Relu,
                                         bias=be1_t[:, m:m + 1])
                    e1r.append(r)
                # e2 -> mu, lv psums [64, NV] each
                pmu = pm.tile([128, 512], F32, tag="pbig", name="pmu")
                nc.tensor.matmul(pmu[:PLAT, :NV], we2_t[:, 0, 0, :], e1r[0][:],
                                 start=True, stop=False)
                nc.tensor.matmul(pmu[:PLAT, :NV], we2_t[:, 1, 0, :], e1r[1][:],
                                 start=False, stop=True)
                plv = pm.tile([128, 512], F32, tag="pbig", name="plv")
                nc.tensor.matmul(plv[:PLAT, :NV], we2_t[:, 0, 1, :], e1r[0][:],
                                 start=True, stop=False)
                nc.tensor.matmul(plv[:PLAT, :NV], we2_t[:, 1, 1, :], e1r[1][:],
                                 start=False, stop=True)
                mus = sbB.tile([PLAT, NV], F32, tag="mus")
                nc.scalar.activation(mus[:], pmu[:PLAT, :NV], AF.Identity,
                                     bias=be2_t[:, 0:1])
                # exp(0.5*(lv + b)) = Exp(psum*0.5 + 0.5*b)
                ex = sbB.tile([PLAT, NV], F32, tag="ex")
                nc.scalar.activation(ex[:], plv[:PLAT, :NV], AF.Exp, scale=0.5,
                                     bias=be2h_t[:, 0:1])
                nc.vector.tensor_tensor(ex[:], ex[:], epsT_t[:, cg0:cg0 + NV], OP.mult)
                zT = sbB.tile([PLAT, NV], F32, tag="zT")
                nc.vector.tensor_tensor(zT[:], ex[:], mus[:], OP.add)
                # dec + r1 accumulation
                psr = [pr.tile([128, 512], F32, tag=f"pr1_{m}", name=f"pr1_{m}") for m in range(2)]
                for kk in range(38):
                    ph = pm.tile([128, 512], F32, tag="pbig")
                    nc.tensor.matmul(ph[:, :NV], wdec_t[:, kk, :], zT[:],
                                     start=True, stop=True)
                    hrr = sbC.tile([128, NV], F32, tag="hrr")
                    nc.scalar.activation(hrr[:], ph[:, :NV], AF.Identity,
                                         bias=bdec_t[:, kk:kk + 1])
                    wkt = wk.tile([128, H], F32, tag="wk2")
                    nc.sync.dma_start(wkt[:], d_wr1[:, kk, :])
                    for m in range(2):
                        nc.tensor.matmul(psr[m][:, :NV], wkt[:, m * 128:(m + 1) * 128],
                                         hrr[:], start=(kk == 0), stop=(kk == 37))
                r1r = []
                for m in range(2):
                    r = sbB.tile([128, NV], F32, tag=f"r1r{m}")
                    nc.scalar.activation(r[:], psr[m][:, :NV], AF.Relu,
                                         bias=br1_t[:, m:m + 1])
                    r1r.append(r)
                ps = pm.tile([128, 512], F32, tag="pbig")
                nc.tensor.matmul(ps[:38, :NV], wr2_t[:, 0, :], r1r[0][:],
                                 start=True, stop=False)
                nc.tensor.matmul(ps[:38, :NV], wr2_t[:, 1, :], r1r[1][:],
                                 start=False, stop=True)
                predT = sbB.tile([38, NV], F32, tag="predT")
                nc.scalar.activation(predT[:], ps[:38, :NV], AF.Identity, bias=br2_t[:])
                # transpose out and DMA
                for off in range(0, GC, 128):
                    w = min(128, GC - off)
                    p2 = pt.tile([128, RF], F32, tag="ptrans")
                    nc.tensor.transpose(p2[:w, :38], predT[:, off:off + w],
                                        ident[:38, :38])
                    ob = sbC.tile([128, 38], F32, tag="ob")
                    nc.scalar.copy(ob[:w, :], p2[:w, :38])
                    nc.sync.dma_start(d_y[cg0 + off:cg0 + off + w, :], ob[:w, :])

            for (cg0, cg1, tl) in chunks:
                for tinfo in tl:
                    process_tile(tinfo, cg0)
                if stages < 4:
                    if stages == 3 and sub >= 5:
                        nc.sync.dma_start(
                            d_y[cg0:cg1, :],
                            hfT[:38, 0, 0, 0:cg1 - cg0].rearrange("p g -> g p"))
                    continue
                vae(cg0, cg1)

    nc.finalize()
    return nc


def _host_prep(inputs, bc=BC):
    """Returns (shared weight arrays dict, per-core input dicts list)."""
    f32 = np.float32
    w_init = inputs['w_init'].astype(f32)
    b_init = inputs['b_init'].astype(f32)
    w_gnn = inputs['w_gnn'].astype(f32)
    b_gnn = inputs['b_gnn'].astype(f32)
    ln_g = inputs['ln_g'].astype(f32)
    ln_b = inputs['ln_b'].astype(f32)
    w_qkv = inputs['w_qkv'].astype(f32)
    b_qkv = inputs['b_qkv'].astype(f32)
    w_ao = inputs['w_ao'].astype(f32)
    b_ao = inputs['b_ao'].astype(f32)

    sh = {}
    # layer0: xt1 = [pts|1] @ wc0, wc0 = [w_init^T; b_init] @ W0^T
    wc0 = np.concatenate([w_init.T, b_init[None, :]], 0) @ w_gnn[0].T
    sh['wc0'] = np.ascontiguousarray(wc0, f32)
    # layers 1,2: W~^T = diag(g_{l-1}) W_l^T ; c_l = W_l @ beta_{l-1}
    wga = np.zeros((128, 2, H), f32)
    wgb = np.zeros((128, 2, H), f32)
    cgn = np.zeros((2, H), f32)
    for l in (1, 2):
        wt = (ln_g[l - 1][:, None] * w_gnn[l].T)  # [256(c), 256(o)]
        cgn[l - 1] = w_gnn[l] @ ln_b[l - 1]       # [256]
        wga[:, l - 1, :] = wt[0:128]
        wgb[:, l - 1, :] = wt[128:256]
    sh['wga'] = wga
    sh['wgb'] = wgb
    sh['cgn'] = cgn
    sh['bg3'] = np.ascontiguousarray(b_gnn, f32)
    wq = (ln_g[2][:, None] * w_qkv.T)             # [256, 768]
    cq = w_qkv @ ln_b[2] + b_qkv
    sh['wqa'] = np.ascontiguousarray(wq[0:128], f32)
    sh['wqb'] = np.ascontiguousarray(wq[128:256], f32)
    sh['cq'] = np.ascontiguousarray(cq[None, :], f32)
    sh['wao4'] = np.ascontiguousarray(
        w_ao.T.reshape(4, 64, H).transpose(1, 0, 2), f32)
    sh['cao'] = np.ascontiguousarray(b_ao[None, :], f32)
    # VAE weights
    we1 = inputs['w_e1'].astype(f32)     # [256, 4864]
    sh['we1'] = np.ascontiguousarray(
        we1.T.reshape(38, 128, H).transpose(1, 0, 2), f32)
    sh['be1'] = np.ascontiguousarray(inputs['b_e1'].astype(f32).reshape(2, 128).T)
    we2 = inputs['w_e2'].astype(f32)     # [128, 256]
    # [c(128), half, m2(mu/lv), 64]
    sh['we2'] = np.ascontiguousarray(
        we2.T.reshape(2, 128, 2, 64).transpose(1, 0, 2, 3), f32)
    sh['be2'] = np.ascontiguousarray(
        inputs['b_e2'].astype(f32).reshape(2, 64).T)
    wdec = inputs['w_dec'].astype(f32)   # [4864, 64]
    sh['wdec'] = np.ascontiguousarray(
        wdec.reshape(38, 128, PLAT).transpose(2, 0, 1), f32)
    sh['bdec'] = np.ascontiguousarray(
        inputs['b_dec'].astype(f32).reshape(38, 128).T, f32)
    wr1 = inputs['w_r1'].astype(f32)
    sh['wr1'] = np.ascontiguousarray(
        wr1.T.reshape(38, 128, H).transpose(1, 0, 2), f32)
    sh['br1'] = np.ascontiguousarray(inputs['b_r1'].astype(f32).reshape(2, 128).T)
    wr2 = inputs['w_r2'].astype(f32)     # [38, 256]
    sh['wr2'] = np.ascontiguousarray(
        wr2.T.reshape(2, 128, 38).transpose(1, 0, 2), f32)
    sh['br2'] = inputs['b_r2'].astype(f32).reshape(38, 1)
    # block-diag masks
    bo = np.full((RF, RF), NEG, f32)
    for g in range(GPT):
        bo[g * J:(g + 1) * J, g * J:(g + 1) * J] = 0.0
    sh['bo'] = bo
    bo4 = np.zeros((RF, 512), f32)
    for hh in range(4):
        bo4[:, hh * 128:hh * 128 + RF] = (bo == 0.0)
    sh['bo4'] = bo4

    pts = inputs['points'].astype(f32)
    feat = inputs['img_features'].astype(f32)
    kv = inputs['k_vals']
    eps = inputs['eps'].astype(f32)
    Ba = pts.shape[0]
    kvrem = (kv.astype(f32).reshape(Ba * J, 1)
             - np.arange(8, dtype=f32)[None, :])
    rinv = (1.0 / kv.astype(f32)).reshape(Ba * J, 1)

    per_core = []
    for c in range(Ba // bc):
        g0, g1 = c * bc, (c + 1) * bc
        r0, r1 = g0 * J, g1 * J
        epsT = np.zeros((PLAT, bc + 8), f32)
        epsT[:, :bc] = eps[g0:g1].T
        m = dict(sh)
        m['pts'] = np.ascontiguousarray(pts.reshape(Ba * J, 2)[r0:r1])
        m['feat'] = np.ascontiguousarray(feat.reshape(Ba * J, H)[r0:r1])
        m['kvrem'] = np.ascontiguousarray(kvrem[r0:r1])
        m['rinv'] = np.ascontiguousarray(rinv[r0:r1])
        m['epsT'] = epsT
        per_core.append(m)
    return per_core


def kernel(**inputs):
    key = 'nc'
    if key not in _CACHE:
        _CACHE[key] = build_nc(BC)
    nc = _CACHE[key]
    in_maps = _host_prep(inputs, BC)
    res = run_bass_kernel_spmd(nc, in_maps, core_ids=list(range(NCORES)))
    ys = [res.results[c]['y'] for c in range(NCORES)]
    out = np.concatenate(ys, 0).reshape(B, J, 2)
    return out.astype(np.float32)



# revision 4
# speedup vs baseline: 1.6056x; 1.6056x over previous
"""Trainium2 Bass kernel for nn_DHDN_Dynamic (hypergraph GNN + attention + VAE).

Self-contained: takes FULL inputs as numpy arrays, shards batch over 8
NeuronCores (pure data parallel), runs one Bass/Tile kernel per core,
gathers the full output.
"""
import sys
sys.path.insert(0, '/opt/trn_rl_repo')
import numpy as np

import concourse.bass as bass
from concourse import bacc
import concourse.mybir as mybir
from concourse.tile import TileContext
from concourse.bass_utils import run_bass_kernel_spmd
from concourse.masks import make_identity

F32 = mybir.dt.float32
U32 = mybir.dt.uint32
AF = mybir.ActivationFunctionType
OP = mybir.AluOpType

B, J, H, MAXK, PLAT = 4096, 19, 256, 8, 64
NHEAD, DH = 4, 64
NCORES = 8
BC = B // NCORES          # graphs per core
GPT = 6                   # graphs per (block-diag) tile
RF = GPT * J              # 114 rows per full tile
NEG = -1.0e9

_CACHE = {}


def _tiles(bc):
    """List of (tile_idx, g0, G) covering bc graphs in 6-graph tiles."""
    out = []
    g0 = 0
    t = 0
    while g0 < bc:
        G = min(GPT, bc - g0)
        out.append((t, g0, G))
        g0 += G
        t += 1
    return out


def _chunks(bc):
    """Two chunks of graphs: [(g0, g1, tiles)]"""
    tl = _tiles(bc)
    half = (len(tl) + 1) // 2
    c0 = tl[:half]
    c1 = tl[half:]
    out = []
    for ts_ in (c0, c1):
        if not ts_:
            continue
        g0 = ts_[0][1]
        g1 = ts_[-1][1] + ts_[-1][2]
        out.append((g0, g1, ts_))
    return out


def build_nc(bc=BC, stages=4, sub=9):
    """Build the Bass IR for one core processing bc graphs."""
    nc = bacc.Bacc("TRN2", target_bir_lowering=False)
    R = bc * J  # total rows (tokens)

    # ---------------- DRAM I/O ----------------
    d_pts = nc.dram_tensor("pts", [R, 2], F32, kind="ExternalInput")
    d_feat = nc.dram_tensor("feat", [R, H], F32, kind="ExternalInput")
    d_kvrem = nc.dram_tensor("kvrem", [R, 8], F32, kind="ExternalInput")
    d_rinv = nc.dram_tensor("rinv", [R, 1], F32, kind="ExternalInput")
    d_bo = nc.dram_tensor("bo", [RF, RF], F32, kind="ExternalInput")
    d_bo4 = nc.dram_tensor("bo4", [RF, 512], F32, kind="ExternalInput")
    d_epsT = nc.dram_tensor("epsT", [PLAT, bc + 8], F32, kind="ExternalInput")
    d_wc0 = nc.dram_tensor("wc0", [3, H], F32, kind="ExternalInput")
    d_wga = nc.dram_tensor("wga", [128, 2, H], F32, kind="ExternalInput")   # layers 1,2 half0
    d_wgb = nc.dram_tensor("wgb", [128, 2, H], F32, kind="ExternalInput")   # layers 1,2 half1
    d_cgn = nc.dram_tensor("cgn", [2, H], F32, kind="ExternalInput")
    d_bg3 = nc.dram_tensor("bg3", [3, H], F32, kind="ExternalInput")        # b_gnn rows
    d_wqa = nc.dram_tensor("wqa", [128, 3 * H], F32, kind="ExternalInput")
    d_wqb = nc.dram_tensor("wqb", [128, 3 * H], F32, kind="ExternalInput")
    d_cq = nc.dram_tensor("cq", [1, 3 * H], F32, kind="ExternalInput")
    d_wao4 = nc.dram_tensor("wao4", [64, 4, H], F32, kind="ExternalInput")
    d_cao = nc.dram_tensor("cao", [1, H], F32, kind="ExternalInput")
    d_we1 = nc.dram_tensor("we1", [128, 38, H], F32, kind="ExternalInput")
    d_be1 = nc.dram_tensor("be1", [128, 2], F32, kind="ExternalInput")
    d_we2 = nc.dram_tensor("we2", [128, 2, 2, 64], F32, kind="ExternalInput")
    d_be2 = nc.dram_tensor("be2", [64, 2], F32, kind="ExternalInput")
    d_wdec = nc.dram_tensor("wdec", [PLAT, 38, 128], F32, kind="ExternalInput")
    d_bdec = nc.dram_tensor("bdec", [128, 38], F32, kind="ExternalInput")
    d_wr1 = nc.dram_tensor("wr1", [128, 38, H], F32, kind="ExternalInput")
    d_br1 = nc.dram_tensor("br1", [128, 2], F32, kind="ExternalInput")
    d_wr2 = nc.dram_tensor("wr2", [128, 2, 38], F32, kind="ExternalInput")
    d_br2 = nc.dram_tensor("br2", [38, 1], F32, kind="ExternalInput")
    d_y = nc.dram_tensor("y", [bc, 38], F32, kind="ExternalOutput")

    chunks = _chunks(bc)
    NV = max(g1 - g0 for g0, g1, _ in chunks)  # VAE free dim (padded)
    NV = max(NV, chunks[0][1] - chunks[0][0])

    with TileContext(nc) as tc:
        with tc.tile_pool(name="cst", bufs=1) as cst, \
             tc.tile_pool(name="wk", bufs=4) as wk, \
             tc.tile_pool(name="hbuf", bufs=1) as hbuf, \
             tc.tile_pool(name="io", bufs=3) as io, \
             tc.tile_pool(name="sbA", bufs=3) as sbA, \
             tc.tile_pool(name="sbB", bufs=2) as sbB, \
             tc.tile_pool(name="sbC", bufs=2) as sbC, \
             tc.tile_pool(name="small", bufs=4) as small, \
             tc.tile_pool(name="pm", bufs=3, space="PSUM") as pm, \
             tc.tile_pool(name="pr", bufs=1, space="PSUM") as pr, \
             tc.tile_pool(name="pt", bufs=2, space="PSUM") as pt:

            # ----- constants -----
            ident = cst.tile([128, 128], F32)
            make_identity(nc, ident[:])
            zcol = cst.tile([128, 1], F32)
            nc.vector.memset(zcol[:], 0.0)
            negt = cst.tile([128, 8], F32)
            nc.vector.memset(negt[:], -2.0e9)
            epsc = cst.tile([128, 1], F32)
            nc.vector.memset(epsc[:], 1.0e-5)
            bo_t = cst.tile([RF, RF], F32)
            nc.sync.dma_start(bo_t[:], d_bo[:])
            bo4_t = cst.tile([RF, 512], F32)
            nc.sync.dma_start(bo4_t[:], d_bo4[:])
            wc0_t = cst.tile([3, H], F32)
            nc.sync.dma_start(wc0_t[:], d_wc0[:])
            wga_t = cst.tile([128, 2, H], F32)
            nc.sync.dma_start(wga_t[:], d_wga[:])
            wgb_t = cst.tile([128, 2, H], F32)
            nc.sync.dma_start(wgb_t[:], d_wgb[:])
            cgnb_t = cst.tile([128, 2, H], F32)
            nc.sync.dma_start(cgnb_t[:], d_cgn[None, :, :].to_broadcast([128, 2, H]))
            bg3_t = cst.tile([3, H], F32)
            nc.sync.dma_start(bg3_t[:], d_bg3[:])
            wqa_t = cst.tile([128, 3 * H], F32)
            nc.sync.dma_start(wqa_t[:], d_wqa[:])
            wqb_t = cst.tile([128, 3 * H], F32)
            nc.sync.dma_start(wqb_t[:], d_wqb[:])
            cqb_t = cst.tile([128, 3 * H], F32)
            nc.sync.dma_start(cqb_t[:], d_cq[None, 0, :].to_broadcast([128, 3 * H]))
            wao4_t = cst.tile([64, 4, H], F32)
            nc.sync.dma_start(wao4_t[:], d_wao4[:])
            caob_t = cst.tile([128, H], F32)
            nc.sync.dma_start(caob_t[:], d_cao[None, 0, :].to_broadcast([128, H]))
            we2_t = cst.tile([128, 2, 2, 64], F32)
            nc.sync.dma_start(we2_t[:], d_we2[:])
            be1_t = cst.tile([128, 2], F32)
            nc.sync.dma_start(be1_t[:], d_be1[:])
            be2_t = cst.tile([64, 2], F32)
            nc.sync.dma_start(be2_t[:], d_be2[:])
            be2h_t = cst.tile([64, 1], F32)
            nc.vector.tensor_scalar_mul(be2h_t[:], be2_t[:, 1:2], 0.5)
            bdec_t = cst.tile([128, 38], F32)
            nc.sync.dma_start(bdec_t[:], d_bdec[:])
            br1_t = cst.tile([128, 2], F32)
            nc.sync.dma_start(br1_t[:], d_br1[:])
            wr2_t = cst.tile([128, 2, 38], F32)
            nc.sync.dma_start(wr2_t[:], d_wr2[:])
            br2_t = cst.tile([38, 1], F32)
            nc.sync.dma_start(br2_t[:], d_br2[:])
            epsT_t = cst.tile([PLAT, bc + 8], F32)
            nc.sync.dma_start(epsT_t[:], d_epsT[:])
            wdec_t = cst.tile([PLAT, 38, 128], F32)
            nc.sync.dma_start(wdec_t[:], d_wdec[:])

            # hfT assembly buffer: [128, half, i(19), NV]
            hfT = hbuf.tile([128, 2, J, NV], F32)

            def process_tile(tinfo, cg0):
                t, g0, G = tinfo
                Rt = G * J
                r0 = g0 * J
                bo = bo_t[:Rt, :Rt]

                # --- input DMAs ---
                ft = io.tile([RF, H], F32, tag="feat_in")
                nc.sync.dma_start(ft[:Rt, :], d_feat[r0:r0 + Rt, :])
                ptst = io.tile([RF, 2], F32, tag="pts_in")
                nc.sync.dma_start(ptst[:Rt, :], d_pts[r0:r0 + Rt, :])
                kvt = io.tile([RF, 8], F32, tag="kv_in")
                nc.sync.dma_start(kvt[:Rt, :], d_kvrem[r0:r0 + Rt, :])
                rit = io.tile([RF, 1], F32, tag="ri_in")
                nc.sync.dma_start(rit[:Rt, :], d_rinv[r0:r0 + Rt, :])

                # --- norms: nf = |F|^2 rows, npts = |p|^2 rows ---
                sqsc = sbB.tile([RF, H], F32, tag="sqscratch")
                nf = small.tile([RF, 1], F32, tag="nf")
                nc.scalar.activation(sqsc[:Rt, :], ft[:Rt, :], AF.Square,
                                     accum_out=nf[:Rt, :])
                np_ = small.tile([RF, 1], F32, tag="npts")
                nc.scalar.activation(sqsc[:Rt, 0:2], ptst[:Rt, :], AF.Square,
                                     accum_out=np_[:Rt, :])

                # --- transposes of F (2 chunks), pts, nf, npts ---
                fT = sbA.tile([128, 2, RF], F32, tag="fT")
                for c in range(2):
                    p = pt.tile([128, RF], F32, tag="ptrans")
                    nc.tensor.transpose(p[:, :Rt], ft[:Rt, c * 128:(c + 1) * 128],
                                        ident[:Rt, :Rt])
                    nc.scalar.copy(fT[:, c, :Rt], p[:, :Rt])
                # ptsT_ext [3, Rt]: rows 0:2 = pts^T, row 2 = ones
                ptsT = sbA.tile([3, RF], F32, tag="ptsT")
                nc.vector.memset(ptsT[0:3, :Rt], 1.0)
                p = pt.tile([128, RF], F32, tag="ptrans")
                nc.tensor.transpose(p[:2, :Rt], ptst[:Rt, :], ident[:Rt, :Rt])
                nc.scalar.copy(ptsT[0:2, :Rt], p[:2, :Rt])
                # norm-row pairs built by transposing [Rt, 2] column tiles:
                # cols (0,1)=(1, -nf/2) (2,3)=(-nf/2, 1) (4,5)=(1, -np/2) (6,7)=(-np/2, 1)
                normin = sbA.tile([RF, 8], F32, tag="normin")
                nc.vector.memset(normin[:Rt, :], 1.0)
                nc.vector.tensor_scalar_mul(normin[:Rt, 1:2], nf[:Rt, :], -0.5)
                nc.vector.tensor_scalar_mul(normin[:Rt, 2:3], nf[:Rt, :], -0.5)
                nc.vector.tensor_scalar_mul(normin[:Rt, 5:6], np_[:Rt, :], -0.5)
                nc.vector.tensor_scalar_mul(normin[:Rt, 6:7], np_[:Rt, :], -0.5)
                nrows = sbA.tile([2, 4, RF], F32, tag="nrows")
                for jj in range(4):
                    p = pt.tile([128, RF], F32, tag="ptrans")
                    nc.tensor.transpose(p[:2, :Rt], normin[:Rt, 2 * jj:2 * jj + 2],
                                        ident[:Rt, :Rt])
                    nc.scalar.copy(nrows[:, jj, :Rt], p[:2, :Rt])

                # --- Gram(feat) - 0.5 nf_i - 0.5 nf_j  (= -0.5 * d2f) ---
                gm = pm.tile([128, 512], F32, tag="pbig")
                nc.tensor.matmul(gm[:Rt, :Rt], fT[:, 0, :Rt], fT[:, 0, :Rt],
                                 start=True, stop=False)
                nc.tensor.matmul(gm[:Rt, :Rt], fT[:, 1, :Rt], fT[:, 1, :Rt],
                                 start=False, stop=False)
                nc.tensor.matmul(gm[:Rt, :Rt], nrows[:, 0, :Rt], nrows[:, 1, :Rt],
                                 start=False, stop=True)
                gmin = sbB.tile([RF, RF], F32, tag="gmin")
                nc.vector.tensor_scalar(gmin[:Rt, :Rt], gm[:Rt, :Rt], 0.0, None, OP.min)
                sf = sbB.tile([RF, RF], F32, tag="sf")
                nc.scalar.activation(sf[:Rt, :Rt], gmin[:Rt, :Rt], AF.Sqrt, scale=-2.0)
                # --- Gram(pts) ---
                gp = pm.tile([128, 512], F32, tag="pbig")
                nc.tensor.matmul(gp[:Rt, :Rt], ptsT[0:2, :Rt], ptsT[0:2, :Rt],
                                 start=True, stop=False)
                nc.tensor.matmul(gp[:Rt, :Rt], nrows[:, 2, :Rt], nrows[:, 3, :Rt],
                                 start=False, stop=True)
                nc.vector.tensor_scalar(gmin[:Rt, :Rt], gp[:Rt, :Rt], 0.0, None, OP.min)
                sp = sbB.tile([RF, RF], F32, tag="sp")
                nc.scalar.activation(sp[:Rt, :Rt], gmin[:Rt, :Rt], AF.Sqrt, scale=-2.0)

                # score = BO - (sf + sp)   (on-block: -(df+dp); off-block <= -1e9)
                nc.vector.tensor_tensor(sf[:Rt, :Rt], sf[:Rt, :Rt], sp[:Rt, :Rt], OP.add)
                score = sbB.tile([RF, RF], F32, tag="score")
                nc.vector.tensor_tensor(score[:Rt, :Rt], bo[:Rt, :Rt], sf[:Rt, :Rt],
                                        OP.subtract)

                # --- top-kv mask S ---
                mx = small.tile([RF, 8], F32, tag="mx")
                nc.vector.max(out=mx[:Rt, :], in_=score[:Rt, :Rt])
                done = small.tile([RF, 8], U32, tag="done")
                nc.vector.tensor_scalar(done[:Rt, :], kvt[:Rt, :], 0.0, None, OP.is_le)
                nc.vector.copy_predicated(mx[:Rt, :], done[:Rt, :], negt[:Rt, :])
                rep = sbB.tile([RF, RF], F32, tag="rep")
                nc.vector.match_replace(out=rep[:Rt, :Rt], in_to_replace=mx[:Rt, :],
                                        in_values=score[:Rt, :Rt], imm_value=NEG)
                S = sbA.tile([RF, RF], F32, tag="S")
                nc.vector.tensor_tensor(S[:Rt, :Rt], score[:Rt, :Rt], rep[:Rt, :Rt],
                                        OP.is_gt)

                # --- A matrix: Af [Rt+1, Rt], rows 0:Rt = (diag(Dinv) Araw)^T, last = 1
                SR = sbB.tile([RF, RF + 1], F32, tag="SR")
                nc.vector.tensor_scalar(SR[:Rt, :Rt], S[:Rt, :Rt], rit[:Rt, :], None,
                                        OP.mult)
                nc.vector.memset(SR[:Rt, Rt:Rt + 1], 1.0)
                araw = pm.tile([128, 512], F32, tag="pbig")
                nc.tensor.matmul(araw[:Rt, :Rt + 1], S[:Rt, :Rt], SR[:Rt, :Rt + 1],
                                 start=True, stop=True)
                dinv = small.tile([RF, 1], F32, tag="dinv")
                nc.vector.reciprocal(dinv[:Rt, :], araw[:Rt, Rt:Rt + 1])
                dz = small.tile([RF, 1], U32, tag="dz")
                nc.vector.tensor_scalar(dz[:Rt, :], araw[:Rt, Rt:Rt + 1], 0.0, None,
                                        OP.is_le)
                nc.vector.copy_predicated(dinv[:Rt, :], dz[:Rt, :], zcol[:Rt, :])
                asc = sbB.tile([RF, RF + 1], F32, tag="asc")
                nc.vector.tensor_scalar(asc[:Rt, :Rt], araw[:Rt, :Rt], dinv[:Rt, :],
                                        None, OP.mult)
                nc.vector.memset(asc[:Rt, Rt:Rt + 1], 1.0)
                pA = pt.tile([128, RF], F32, tag="ptrans")
                nc.tensor.transpose(pA[:Rt + 1, :Rt], asc[:Rt, :Rt + 1],
                                    ident[:Rt, :Rt])
                Af = sbA.tile([RF + 1, RF], F32, tag="Af")
                nc.scalar.copy(Af[:Rt + 1, :Rt], pA[:Rt + 1, :Rt])

                if stages < 2:
                    nc.sync.dma_start(d_y[g0:g0 + G, :],
                                      Af[:G, :38])
                    return
                # ---------------- 3 hconv layers ----------------
                h = None
                for l in range(3):
                    xt = pm.tile([128, 512], F32, tag="pbig")
                    if l == 0:
                        nc.tensor.matmul(xt[:Rt, :H], ptsT[:, :Rt], wc0_t[:],
                                         start=True, stop=True)
                    else:
                        hT = sbC.tile([128, 2, RF], F32, tag="hT")
                        for c in range(2):
                            p2 = pt.tile([128, RF], F32, tag="ptrans")
                            nc.tensor.transpose(p2[:, :Rt],
                                                h[:Rt, c * 128:(c + 1) * 128],
                                                ident[:Rt, :Rt])
                            nc.scalar.copy(hT[:, c, :Rt], p2[:, :Rt])
                        wl = l - 1
                        nc.tensor.matmul(xt[:Rt, :H], hT[:, 0, :Rt],
                                         wga_t[:, wl, :], start=True, stop=False)
                        nc.tensor.matmul(xt[:Rt, :H], hT[:, 1, :Rt],
                                         wgb_t[:, wl, :], start=False, stop=True)
                    xts = sbC.tile([RF + 1, H], F32, tag="xts")
                    if l == 0:
                        nc.vector.tensor_copy(xts[:Rt, :], xt[:Rt, :H])
                    else:
                        nc.vector.tensor_tensor(xts[:Rt, :], xt[:Rt, :H],
                                                cgnb_t[:Rt, l - 1, :], OP.add)
                    nc.sync.dma_start(xts[Rt:Rt + 1, :], d_bg3[l:l + 1, :])
                    agg = pm.tile([128, 512], F32, tag="pbig")
                    nc.tensor.matmul(agg[:Rt, :H], Af[:Rt + 1, :Rt], xts[:Rt + 1, :],
                                     start=True, stop=True)
                    # relu + LN stats
                    hr = sbC.tile([RF, H], F32, tag="hrelu")
                    rsum = small.tile([RF, 1], F32, tag="rsum")
                    nc.scalar.activation(hr[:Rt, :], agg[:Rt, :H], AF.Relu,
                                         accum_out=rsum[:Rt, :])
                    ssq = small.tile([RF, 1], F32, tag="ssq")
                    nc.scalar.activation(sqsc[:Rt, :], hr[:Rt, :], AF.Square,
                                         accum_out=ssq[:Rt, :])
                    mu = small.tile([RF, 1], F32, tag="mu")
                    nc.vector.tensor_scalar_mul(mu[:Rt, :], rsum[:Rt, :], 1.0 / H)
                    var = small.tile([RF, 1], F32, tag="var")
                    nc.vector.tensor_scalar_mul(var[:Rt, :], ssq[:Rt, :], 1.0 / H)
                    mu2 = small.tile([RF, 1], F32, tag="mu2")
                    nc.vector.tensor_tensor(mu2[:Rt, :], mu[:Rt, :], mu[:Rt, :], OP.mult)
                    nc.vector.tensor_tensor(var[:Rt, :], var[:Rt, :], mu2[:Rt, :],
                                            OP.subtract)
                    sg = small.tile([RF, 1], F32, tag="sg")
                    nc.scalar.activation(sg[:Rt, :], var[:Rt, :], AF.Sqrt, bias=epsc[:Rt, :])
                    rs = small.tile([RF, 1], F32, tag="rs")
                    nc.vector.reciprocal(rs[:Rt, :], sg[:Rt, :])
                    h = sbC.tile([RF, H], F32, tag=f"h{l}")
                    nc.vector.tensor_scalar(h[:Rt, :], hr[:Rt, :], mu[:Rt, :],
                                            rs[:Rt, :], OP.subtract, OP.mult)

                if stages < 3:
                    nc.sync.dma_start(d_y[g0:g0 + G, :], h[:G, :38])
                    return
                # ---------------- attention ----------------
                hT = sbC.tile([128, 2, RF], F32, tag="hT")
                for c in range(2):
                    p2 = pt.tile([128, RF], F32, tag="ptrans")
                    nc.tensor.transpose(p2[:, :Rt], h[:Rt, c * 128:(c + 1) * 128],
                                        ident[:Rt, :Rt])
                    nc.scalar.copy(hT[:, c, :Rt], p2[:, :Rt])
                qkvs = sbB.tile([RF, 3 * H], F32, tag="qkvs")
                for nh in range(2):
                    qkv = pm.tile([128, 512], F32, tag="pbig")
                    s0, s1 = nh * 384, (nh + 1) * 384
                    nc.tensor.matmul(qkv[:Rt, :384], hT[:, 0, :Rt],
                                     wqa_t[:, s0:s1], start=True, stop=False)
                    nc.tensor.matmul(qkv[:Rt, :384], hT[:, 1, :Rt],
                                     wqb_t[:, s0:s1], start=False, stop=True)
                    nc.vector.tensor_tensor(qkvs[:Rt, s0:s1], qkv[:Rt, :384],
                                            cqb_t[:Rt, s0:s1], OP.add)
                if sub < 1:
                    nc.sync.dma_start(d_y[g0:g0 + G, :], qkvs[:G, :38])
                    return
                # q,k transposed per head-pair; q scaled by 1/8
                qT = sbA.tile([64, 4, RF], F32, tag="qT")
                kT = sbA.tile([64, 4, 128], F32, tag="kT")
                if Rt < 128:
                    nc.vector.memset(kT[:, :, Rt:128], 0.0)
                for hh in range(4):
                    p2 = pt.tile([128, RF], F32, tag="ptrans")
                    nc.tensor.transpose(p2[:64, :Rt], qkvs[:Rt, hh * 64:(hh + 1) * 64],
                                        ident[:Rt, :Rt])
                    nc.scalar.mul(qT[:, hh, :Rt], p2[:64, :Rt], 0.125)
                    p2 = pt.tile([128, RF], F32, tag="ptrans")
                    nc.tensor.transpose(p2[:64, :Rt],
                                        qkvs[:Rt, H + hh * 64:H + (hh + 1) * 64],
                                        ident[:Rt, :Rt])
                    nc.scalar.copy(kT[:, hh, :Rt], p2[:64, :Rt])
                if sub < 2:
                    nc.sync.dma_start(d_y[g0:g0 + G, :], qT[:G, 0, :38])
                    return
                # scores for 4 heads into one psum [Rt, 4*Rt]
                scs = []
                for hh in range(4):
                    sc = pm.tile([128, 512], F32, tag="pbig", name=f"sc{hh}")
                    nc.tensor.matmul(sc[:Rt, :128], qT[:, hh, :Rt], kT[:, hh, :],
                                     start=True, stop=True)
                    scs.append(sc)
                if sub == 2.01:
                    tmpo = sbB.tile([RF, 4 * RF], F32, tag="tmpo")
                    nc.vector.tensor_copy(tmpo[:Rt, :4 * Rt], sc[:Rt, :4 * Rt])
                    nc.sync.dma_start(d_y[g0:g0 + G, :], tmpo[:G, :38])
                    return
                pexp = sbB.tile([RF, 512], F32, tag="pexp")
                for hh in range(4):
                    nc.scalar.activation(pexp[:Rt, hh * 128:(hh + 1) * 128],
                                         scs[hh][:Rt, :128], AF.Exp)
                # mask off-block + per-head row sums
                if sub == 2.02:
                    nc.sync.dma_start(d_y[g0:g0 + G, :], pexp[:G, :38])
                    return
                sums = small.tile([RF, 4], F32, tag="sums")
                pm4 = sbB.tile([RF, 512], F32, tag="pm4")
                nc.vector.tensor_tensor(
                    pm4[:Rt, :].rearrange("p (h j) -> p h j", h=4),
                    pexp[:Rt, :].rearrange("p (h j) -> p h j", h=4),
                    bo4_t[:Rt, :].rearrange("p (h j) -> p h j", h=4), OP.mult)
                nc.vector.tensor_reduce(
                    sums[:Rt, :],
                    pm4[:Rt, :].rearrange("p (h j) -> p h j", h=4),
                    axis=mybir.AxisListType.X, op=OP.add)
                if sub == 2.03:
                    nc.sync.dma_start(d_y[g0:g0 + G, :], pm4[:G, :38])
                    return
                rsum4 = small.tile([RF, 4], F32, tag="rsum4")
                nc.vector.reciprocal(rsum4[:Rt, :], sums[:Rt, :])
                if sub == 2.04:
                    nc.sync.dma_start(d_y[g0:g0 + G, :], sums[:G, :4])
                    return
                att = sbB.tile([RF, 512], F32, tag="att")
                nc.vector.tensor_tensor(
                    att[:Rt, :].rearrange("p (h j) -> p h j", h=4),
                    pm4[:Rt, :].rearrange("p (h j) -> p h j", h=4),
                    rsum4[:Rt, :, None].to_broadcast((Rt, 4, 128)),
                    OP.mult)
                if sub < 3:
                    nc.sync.dma_start(d_y[g0:g0 + G, :], att[:G, :38])
                    return
                # attT per head, then AV; oT4 [64, 4, RF]
                oT = sbA.tile([64, 4, RF], F32, tag="oT")
                for hh in range(4):
                    pa = pt.tile([128, RF], F32, tag="ptrans")
                    nc.tensor.transpose(pa[:Rt, :Rt],
                                        att[:Rt, hh * 128:hh * 128 + Rt],
                                        ident[:Rt, :Rt])
                    aT = sbB.tile([RF, RF], F32, tag="aT")
                    nc.scalar.copy(aT[:Rt, :Rt], pa[:Rt, :Rt])
                    po = pm.tile([128, 512], F32, tag="pbig", name=f"po{hh}")
                    nc.tensor.matmul(po[:64, :Rt], qkvs[:Rt, 512 + hh * 64:512 + (hh + 1) * 64],
                                     aT[:Rt, :Rt], start=True, stop=True)
                    nc.scalar.copy(oT[:, hh, :Rt], po[:64, :Rt])
                if sub < 4:
                    nc.sync.dma_start(d_y[g0:g0 + G, :], oT[:G, 0, :38])
                    return
                hat = pm.tile([128, 512], F32, tag="pbig")
                for hh in range(4):
                    nc.tensor.matmul(hat[:Rt, :H], oT[:, hh, :Rt], wao4_t[:, hh, :],
                                     start=(hh == 0), stop=(hh == 3))
                hats = sbC.tile([RF, H], F32, tag="hats")
                nc.vector.tensor_tensor(hats[:Rt, :], hat[:Rt, :H],
                                        caob_t[:Rt, :], OP.add)
                # scatter transposed into hfT
                col0 = g0 - cg0
                for c in range(2):
                    p2 = pt.tile([128, RF], F32, tag="ptrans")
                    nc.tensor.transpose(p2[:, :Rt], hats[:Rt, c * 128:(c + 1) * 128],
                                        ident[:Rt, :Rt])
                    dst = hfT[:, c, :, col0:col0 + G].rearrange("p i g -> p g i")
                    nc.vector.tensor_copy(dst, p2[:, :Rt].rearrange(
                        "p (g i) -> p g i", i=J))

            def vae(cg0, cg1):
                GC = cg1 - cg0
                if GC < NV:
                    nc.vector.memset(hfT[:, :, :, GC:NV], 0.0)
                # e1: out e1r [2][128, NV]; stream we1 per-kk
                pse = [pr.tile([128, 512], F32, tag=f"pr1_{m}", name=f"pse{m}")
                       for m in range(2)]
                for kk in range(38):
                    i, half = kk // 2, kk % 2
                    wkt = wk.tile([128, H], F32, tag="wk1")
                    nc.sync.dma_start(wkt[:], d_we1[:, kk, :])
                    for m in range(2):
                        nc.tensor.matmul(pse[m][:, :NV], wkt[:, m * 128:(m + 1) * 128],
                                         hfT[:, half, i, :], start=(kk == 0),
                                         stop=(kk == 37))
                e1r = []
                for m in range(2):
                    r = sbB.tile([128, NV], F32, tag=f"e1r{m}", name=f"e1r{m}")
                    nc.scalar.activation(r[:], pse[m][:, :NV], AF.Relu,
                                         bias=be1_t[:, m:m + 1])
                    e1r.append(r)
                # e2 -> mu, lv psums [64, NV] each
                pmu = pm.tile([128, 512], F32, tag="pbig", name="pmu")
                nc.tensor.matmul(pmu[:PLAT, :NV], we2_t[:, 0, 0, :], e1r[0][:],
                                 start=True, stop=False)
                nc.tensor.matmul(pmu[:PLAT, :NV], we2_t[:, 1, 0, :], e1r[1][:],
                                 start=False, stop=True)
                plv = pm.tile([128, 512], F32, tag="pbig", name="plv")
                nc.tensor.matmul(plv[:PLAT, :NV], we2_t[:, 0, 1, :], e1r[0][:],
                                 start=True, stop=False)
                nc.tensor.matmul(plv[:PLAT, :NV], we2_t[:, 1, 1, :], e1r[1][:],
                                 start=False, stop=True)
                mus = sbB.tile([PLAT, NV], F32, tag="mus")
                nc.scalar.activation(mus[:], pmu[:PLAT, :NV], AF.Identity,
                                     bias=be2_t[:, 0:1])
                # exp(0.5*(lv + b)) = Exp(psum*0.5 + 0.5*b)
                ex = sbB.tile([PLAT, NV], F32, tag="ex")
                nc.scalar.activation(ex[:], plv[:PLAT, :NV], AF.Exp, scale=0.5,
                                     bias=be2h_t[:, 0:1])
                nc.vector.tensor_tensor(ex[:], ex[:], epsT_t[:, cg0:cg0 + NV], OP.mult)
                zT = sbB.tile([PLAT, NV], F32, tag="zT")
                nc.vector.tensor_tensor(zT[:], ex[:], mus[:], OP.add)
                # dec + r1 accumulation
                psr = [pr.tile([128, 512], F32, tag=f"pr1_{m}", name=f"pr1_{m}") for m in range(2)]
                for kk in range(38):
                    ph = pm.tile([128, 512], F32, tag="pbig")
                    nc.tensor.matmul(ph[:, :NV], wdec_t[:, kk, :], zT[:],
                                     start=True, stop=True)
                    hrr = sbC.tile([128, NV], F32, tag="hrr")
                    nc.scalar.activation(hrr[:], ph[:, :NV], AF.Identity,
                                         bias=bdec_t[:, kk:kk + 1])
                    wkt = wk.tile([128, H], F32, tag="wk2")
                    nc.sync.dma_start(wkt[:], d_wr1[:, kk, :])
                    for m in range(2):
                        nc.tensor.matmul(psr[m][:, :NV], wkt[:, m * 128:(m + 1) * 128],
                                         hrr[:], start=(kk == 0), stop=(kk == 37))
                r1r = []
                for m in range(2):
                    r = sbB.tile([128, NV], F32, tag=f"r1r{m}")
                    nc.scalar.activation(r[:], psr[m][:, :NV], AF.Relu,
                                         bias=br1_t[:, m:m + 1])
                    r1r.append(r)
                ps = pm.tile([128, 512], F32, tag="pbig")
                nc.tensor.matmul(ps[:38, :NV], wr2_t[:, 0, :], r1r[0][:],
                                 start=True, stop=False)
                nc.tensor.matmul(ps[:38, :NV], wr2_t[:, 1, :], r1r[1][:],
                                 start=False, stop=True)
                predT = sbB.tile([38, NV], F32, tag="predT")
                nc.scalar.activation(predT[:], ps[:38, :NV], AF.Identity, bias=br2_t[:])
                # transpose out and DMA
                for off in range(0, GC, 128):
                    w = min(128, GC - off)
                    p2 = pt.tile([128, RF], F32, tag="ptrans")
                    nc.tensor.transpose(p2[:w, :38], predT[:, off:off + w],
                                        ident[:38, :38])
                    ob = sbC.tile([128, 38], F32, tag="ob")
                    nc.scalar.copy(ob[:w, :], p2[:w, :38])
                    nc.sync.dma_start(d_y[cg0 + off:cg0 + off + w, :], ob[:w, :])

            for (cg0, cg1, tl) in chunks:
                for tinfo in tl:
                    process_tile(tinfo, cg0)
                if stages < 4:
                    if stages == 3 and sub >= 5:
                        nc.sync.dma_start(
                            d_y[cg0:cg1, :],
                            hfT[:38, 0, 0, 0:cg1 - cg0].rearrange("p g -> g p"))
                    continue
                vae(cg0, cg1)

    nc.finalize()
    return nc


def _host_prep(inputs, bc=BC):
    """Returns (shared weight arrays dict, per-core input dicts list)."""
    f32 = np.float32
    w_init = inputs['w_init'].astype(f32)
    b_init = inputs['b_init'].astype(f32)
    w_gnn = inputs['w_gnn'].astype(f32)
    b_gnn = inputs['b_gnn'].astype(f32)
    ln_g = inputs['ln_g'].astype(f32)
    ln_b = inputs['ln_b'].astype(f32)
    w_qkv = inputs['w_qkv'].astype(f32)
    b_qkv = inputs['b_qkv'].astype(f32)
    w_ao = inputs['w_ao'].astype(f32)
    b_ao = inputs['b_ao'].astype(f32)

    sh = {}
    # layer0: xt1 = [pts|1] @ wc0, wc0 = [w_init^T; b_init] @ W0^T
    wc0 = np.concatenate([w_init.T, b_init[None, :]], 0) @ w_gnn[0].T
    sh['wc0'] = np.ascontiguousarray(wc0, f32)
    # layers 1,2: W~^T = diag(g_{l-1}) W_l^T ; c_l = W_l @ beta_{l-1}
    wga = np.zeros((128, 2, H), f32)
    wgb = np.zeros((128, 2, H), f32)
    cgn = np.zeros((2, H), f32)
    for l in (1, 2):
        wt = (ln_g[l - 1][:, None] * w_gnn[l].T)  # [256(c), 256(o)]
        cgn[l - 1] = w_gnn[l] @ ln_b[l - 1]       # [256]
        wga[:, l - 1, :] = wt[0:128]
        wgb[:, l - 1, :] = wt[128:256]
    sh['wga'] = wga
    sh['wgb'] = wgb
    sh['cgn'] = cgn
    sh['bg3'] = np.ascontiguousarray(b_gnn, f32)
    wq = (ln_g[2][:, None] * w_qkv.T)             # [256, 768]
    cq = w_qkv @ ln_b[2] + b_qkv
    sh['wqa'] = np.ascontiguousarray(wq[0:128], f32)
    sh['wqb'] = np.ascontiguousarray(wq[128:256], f32)
    sh['cq'] = np.ascontiguousarray(cq[None, :], f32)
    sh['wao4'] = np.ascontiguousarray(
        w_ao.T.reshape(4, 64, H).transpose(1, 0, 2), f32)
    sh['cao'] = np.ascontiguousarray(b_ao[None, :], f32)
    # VAE weights
    we1 = inputs['w_e1'].astype(f32)     # [256, 4864]
    sh['we1'] = np.ascontiguousarray(
        we1.T.reshape(38, 128, H).transpose(1, 0, 2), f32)
    sh['be1'] = np.ascontiguousarray(inputs['b_e1'].astype(f32).reshape(2, 128).T)
    we2 = inputs['w_e2'].astype(f32)     # [128, 256]
    # [c(128), half, m2(mu/lv), 64]
    sh['we2'] = np.ascontiguousarray(
        we2.T.reshape(2, 128, 2, 64).transpose(1, 0, 2, 3), f32)
    sh['be2'] = np.ascontiguousarray(
        inputs['b_e2'].astype(f32).reshape(2, 64).T)
    wdec = inputs['w_dec'].astype(f32)   # [4864, 64]
    sh['wdec'] = np.ascontiguousarray(
        wdec.reshape(38, 128, PLAT).transpose(2, 0, 1), f32)
    sh['bdec'] = np.ascontiguousarray(
        inputs['b_dec'].astype(f32).reshape(38, 128).T, f32)
    wr1 = inputs['w_r1'].astype(f32)
    sh['wr1'] = np.ascontiguousarray(
        wr1.T.reshape(38, 128, H).transpose(1, 0, 2), f32)
    sh['br1'] = np.ascontiguousarray(inputs['b_r1'].astype(f32).reshape(2, 128).T)
    wr2 = inputs['w_r2'].astype(f32)     # [38, 256]
    sh['wr2'] = np.ascontiguousarray(
        wr2.T.reshape(2, 128, 38).transpose(1, 0, 2), f32)
    sh['br2'] = inputs['b_r2'].astype(f32).reshape(38, 1)
    # block-diag masks
    bo = np.full((RF, RF), NEG, f32)
    for g in range(GPT):
        bo[g * J:(g + 1) * J, g * J:(g + 1) * J] = 0.0
    sh['bo'] = bo
    bo4 = np.zeros((RF, 512), f32)
    for hh in range(4):
        bo4[:, hh * 128:hh * 128 + RF] = (bo == 0.0)
    sh['bo4'] = bo4

    pts = inputs['points'].astype(f32)
    feat = inputs['img_features'].astype(f32)
    kv = inputs['k_vals']
    eps = inputs['eps'].astype(f32)
    Ba = pts.shape[0]
    kvrem = (kv.astype(f32).reshape(Ba * J, 1)
             - np.arange(8, dtype=f32)[None, :])
    rinv = (1.0 / kv.astype(f32)).reshape(Ba * J, 1)

    per_core = []
    for c in range(Ba // bc):
        g0, g1 = c * bc, (c + 1) * bc
        r0, r1 = g0 * J, g1 * J
        epsT = np.zeros((PLAT, bc + 8), f32)
        epsT[:, :bc] = eps[g0:g1].T
        m = dict(sh)
        m['pts'] = np.ascontiguousarray(pts.reshape(Ba * J, 2)[r0:r1])
        m['feat'] = np.ascontiguousarray(feat.reshape(Ba * J, H)[r0:r1])
        m['kvrem'] = np.ascontiguousarray(kvrem[r0:r1])
        m['rinv'] = np.ascontiguousarray(rinv[r0:r1])
        m['epsT'] = epsT
        per_core.append(m)
    return per_core


def kernel(**inputs):
    key = 'nc'
    if key not in _CACHE:
        _CACHE[key] = build_nc(BC)
    nc = _CACHE[key]
    in_maps = _host_prep(inputs, BC)
    res = run_bass_kernel_spmd(nc, in_maps, core_ids=list(range(NCORES)))
    ys = [res.results[c]['y'] for c in range(NCORES)]
    out = np.concatenate(ys, 0).reshape(B, J, 2)
    return out.astype(np.float32)



# revision 5
# speedup vs baseline: 1.6073x; 1.0010x over previous
"""Trainium2 Bass kernel v2 for nn_DHDN_Dynamic (hypergraph GNN + attn + VAE).

bf16 matmul path everywhere except the distance/top-k selection (fp32).
Folds: ln into weights, w_ao into w_e1, w_r1@w_dec into one matmul,
biases via rank-1 matmuls, block-mask via low-rank matmul, sqrt via
exp(ln/2) so the scalar engine keeps a single activation table.
"""
import sys
sys.path.insert(0, '/opt/trn_rl_repo')
import numpy as np
import ml_dtypes

import concourse.bass as bass
from concourse import bacc
import concourse.mybir as mybir
from concourse.tile import TileContext
from concourse.bass_utils import run_bass_kernel_spmd
from concourse.masks import make_identity

F32 = mybir.dt.float32
BF16 = mybir.dt.bfloat16
U32 = mybir.dt.uint32
AF = mybir.ActivationFunctionType
OP = mybir.AluOpType
BF = ml_dtypes.bfloat16

B, J, H, MAXK, PLAT = 4096, 19, 256, 8, 64
NHEAD, DH = 4, 64
NCORES = 8
BC = B // NCORES
GPT = 6
RF = GPT * J          # 114
NEG = -1.0e9

_CACHE = {}


def _tiles(bc):
    out = []
    g0 = 0
    t = 0
    while g0 < bc:
        G = min(GPT, bc - g0)
        out.append((t, g0, G))
        g0 += G
        t += 1
    return out


def _chunks(bc):
    tl = _tiles(bc)
    half = (len(tl) + 1) // 2
    out = []
    for ts_ in (tl[:half], tl[half:]):
        if not ts_:
            continue
        g0 = ts_[0][1]
        g1 = ts_[-1][1] + ts_[-1][2]
        out.append((g0, g1, ts_))
    return out


def build_nc(bc=BC):
    nc = bacc.Bacc("TRN2", target_bir_lowering=False)
    R = bc * J

    chunks = _chunks(bc)
    NV = max(g1 - g0 for g0, g1, _ in chunks)

    # ---------------- DRAM I/O ----------------
    d_feat = nc.dram_tensor("feat", [R, H], F32, kind="ExternalInput")
    d_aux = nc.dram_tensor("aux", [R, 11], F32, kind="ExternalInput")
    d_bo = nc.dram_tensor("bo", [RF, RF], F32, kind="ExternalInput")
    d_sel = nc.dram_tensor("sel", [5, 2, RF], F32, kind="ExternalInput")
    d_EL = nc.dram_tensor("EL", [7, RF], BF16, kind="ExternalInput")
    d_EM = nc.dram_tensor("EM", [7, 512], BF16, kind="ExternalInput")
    d_cmb = nc.dram_tensor("cmb", [1, 3, H], BF16, kind="ExternalInput")
    d_wc0 = nc.dram_tensor("wc0", [3, H], BF16, kind="ExternalInput")
    d_wg = nc.dram_tensor("wg", [128, 2, 2, H], BF16, kind="ExternalInput")
    d_wqk = nc.dram_tensor("wqk", [128, 2, 4, 128], BF16, kind="ExternalInput")
    d_wv = nc.dram_tensor("wv", [128, 2, H], BF16, kind="ExternalInput")
    d_cqc = nc.dram_tensor("cqc", [1, 2, 128], BF16, kind="ExternalInput")
    d_cvr = nc.dram_tensor("cvr", [1, H], BF16, kind="ExternalInput")
    d_we1 = nc.dram_tensor("we1", [128, 38, H], BF16, kind="ExternalInput")
    d_be1 = nc.dram_tensor("be1", [128, 2], F32, kind="ExternalInput")
    d_we2 = nc.dram_tensor("we2", [128, 2, 128], BF16, kind="ExternalInput")
    d_be2 = nc.dram_tensor("be2", [64, 2], F32, kind="ExternalInput")
    d_wrd = nc.dram_tensor("wrd", [64, 2, 128], BF16, kind="ExternalInput")
    d_brd = nc.dram_tensor("brd", [128, 2], F32, kind="ExternalInput")
    d_wr2 = nc.dram_tensor("wr2", [128, 2, 38], BF16, kind="ExternalInput")
    d_br2 = nc.dram_tensor("br2", [38, 1], F32, kind="ExternalInput")
    d_epsT = nc.dram_tensor("epsT", [PLAT, bc + 8], F32, kind="ExternalInput")
    d_y = nc.dram_tensor("y", [bc, 38], F32, kind="ExternalOutput")

    with TileContext(nc) as tc:
        with tc.tile_pool(name="cst", bufs=1) as cst, \
             tc.tile_pool(name="hbuf", bufs=2) as hbuf, \
             tc.tile_pool(name="io", bufs=3) as io, \
             tc.tile_pool(name="sbA", bufs=4) as sbA, \
             tc.tile_pool(name="sbB", bufs=4) as sbB, \
             tc.tile_pool(name="sbC", bufs=4) as sbC, \
             tc.tile_pool(name="small", bufs=6) as small, \
             tc.tile_pool(name="pm", bufs=2, space="PSUM") as pm, \
             tc.tile_pool(name="pt", bufs=1, space="PSUM") as pt, \
             tc.tile_pool(name="ptb", bufs=1, space="PSUM") as ptb, \
             tc.tile_pool(name="psc", bufs=2, space="PSUM") as psc, \
             tc.tile_pool(name="pr", bufs=1, space="PSUM") as pr:

            # ---- constants ----
            identf = cst.tile([128, 128], F32)
            make_identity(nc, identf[:])
            identb = cst.tile([128, 128], BF16)
            make_identity(nc, identb[:])
            zcol = cst.tile([128, 1], F32)
            nc.vector.memset(zcol[:], 0.0)
            negt = cst.tile([128, 8], F32)
            nc.vector.memset(negt[:], -2.0e9)
            epsd = cst.tile([128, 1], F32)
            nc.vector.memset(epsd[:], 1.0e-20)
            epsc = cst.tile([128, 1], F32)
            nc.vector.memset(epsc[:], 1.0e-5)
            ones1f = cst.tile([1, RF], F32)
            nc.vector.memset(ones1f[:], 1.0)
            ones1b = cst.tile([1, RF], BF16)
            nc.vector.memset(ones1b[:], 1.0)
            onescol = cst.tile([128, 1], BF16)
            nc.vector.memset(onescol[:], 1.0)
            # K=5 selector rows: pick -nf/2 (row 3) or -np/2 (row 4) of nrm
            sel = cst.tile([5, 2, RF], F32)
            nc.sync.dma_start(sel[:], d_sel[:])

            bo_t = cst.tile([RF, RF], F32)
            nc.sync.dma_start(bo_t[:], d_bo[:])
            ELb = cst.tile([7, RF], BF16)
            nc.sync.dma_start(ELb[:], d_EL[:])
            EMb = cst.tile([7, 512], BF16)
            nc.sync.dma_start(EMb[:], d_EM[:])
            cmbb = cst.tile([1, 3, H], BF16)
            nc.sync.dma_start(cmbb[:], d_cmb[:])
            cqcb = cst.tile([1, 2, 128], BF16)
            nc.sync.dma_start(cqcb[:], d_cqc[:])
            cvrb = cst.tile([1, H], BF16)
            nc.sync.dma_start(cvrb[:], d_cvr[:])
            wc0b = cst.tile([3, H], BF16)
            nc.sync.dma_start(wc0b[:], d_wc0[:])
            wgb = cst.tile([128, 2, 2, H], BF16)
            nc.sync.dma_start(wgb[:], d_wg[:])
            wqkb = cst.tile([128, 2, 4, 128], BF16)
            nc.sync.dma_start(wqkb[:], d_wqk[:])
            wvb = cst.tile([128, 2, H], BF16)
            nc.sync.dma_start(wvb[:], d_wv[:])
            we1b = cst.tile([128, 38, H], BF16)
            nc.sync.dma_start(we1b[:], d_we1[:])
            we2b = cst.tile([128, 2, 128], BF16)
            nc.sync.dma_start(we2b[:], d_we2[:])
            wrdb = cst.tile([64, 2, 128], BF16)
            nc.sync.dma_start(wrdb[:], d_wrd[:])
            wr2b = cst.tile([128, 2, 38], BF16)
            nc.sync.dma_start(wr2b[:], d_wr2[:])
            be1_t = cst.tile([128, 2], F32)
            nc.sync.dma_start(be1_t[:], d_be1[:])
            be2_t = cst.tile([64, 2], F32)
            nc.sync.dma_start(be2_t[:], d_be2[:])
            be2h_t = cst.tile([64, 1], F32)
            nc.vector.tensor_scalar_mul(be2h_t[:], be2_t[:, 1:2], 0.5)
            brd_t = cst.tile([128, 2], F32)
            nc.sync.dma_start(brd_t[:], d_brd[:])
            br2_t = cst.tile([38, 1], F32)
            nc.sync.dma_start(br2_t[:], d_br2[:])
            epsT_t = cst.tile([PLAT, bc + 8], F32)
            nc.sync.dma_start(epsT_t[:], d_epsT[:])

            def process_tile(tinfo, cg0, hfT):
                t, g0, G = tinfo
                Rt = G * J
                r0 = g0 * J
                idf = identf[:Rt, :Rt]
                idb = identb[:Rt, :Rt]

                ft = io.tile([RF, H], F32, tag="ft")
                nc.sync.dma_start(ft[:Rt, :], d_feat[r0:r0 + Rt, :])
                aux = io.tile([RF, 11], F32, tag="aux")
                nc.sync.dma_start(aux[:Rt, :], d_aux[r0:r0 + Rt, :])

                # ---- norms ----
                sqs = sbB.tile([RF, H], BF16, tag="sqs")
                nf = small.tile([RF, 1], F32, tag="nf")
                nc.scalar.activation(sqs[:Rt, :], ft[:Rt, :], AF.Square,
                                     accum_out=nf[:Rt, :])
                np_ = small.tile([RF, 1], F32, tag="np")
                nc.scalar.activation(sqs[:Rt, 0:2], aux[:Rt, 0:2], AF.Square,
                                     accum_out=np_[:Rt, :])
                # ---- fT (fp32, 2 chunks) ----
                fT = sbA.tile([128, 2, RF], F32, tag="fT")
                for c in range(2):
                    p = pt.tile([128, 512], F32, tag="pt")
                    nc.tensor.transpose(p[:, :Rt], ft[:Rt, c * 128:(c + 1) * 128],
                                        idf)
                    if c == 0:
                        nc.vector.tensor_copy(fT[:, c, :Rt], p[:, :Rt])
                    else:
                        nc.scalar.copy(fT[:, c, :Rt], p[:, :Rt])

                # normin cols: 0,1 pts  2 ones  3 -nf/2  4 -np/2
                normin = sbA.tile([RF, 5], F32, tag="normin")
                nc.vector.tensor_copy(normin[:Rt, 0:2], aux[:Rt, 0:2])
                nc.vector.memset(normin[:Rt, 2:3], 1.0)
                nc.vector.tensor_scalar_mul(normin[:Rt, 3:4], nf[:Rt, :], -0.5)
                nc.vector.tensor_scalar_mul(normin[:Rt, 4:5], np_[:Rt, :], -0.5)
                pn = pt.tile([128, 512], F32, tag="pt")
                nc.tensor.transpose(pn[:5, :Rt], normin[:Rt, :], idf)
                nrm = sbA.tile([5, RF], F32, tag="nrm")
                nc.scalar.copy(nrm[:, :Rt], pn[:5, :Rt])
                ptsTb = sbA.tile([3, RF], BF16, tag="ptsTb")
                nc.scalar.copy(ptsTb[:, :Rt], pn[:3, :Rt])

                # ---- feat dist: psum = G - 0.5 nf_j ; d2 = relu(-2x + nf_i) ----
                gm = pm.tile([128, 512], F32, tag="pbig")
                nc.tensor.matmul(gm[:Rt, :Rt], fT[:, 0, :Rt], fT[:, 0, :Rt],
                                 start=True, stop=False)
                nc.tensor.matmul(gm[:Rt, :Rt], fT[:, 1, :Rt], fT[:, 1, :Rt],
                                 start=False, stop=False)
                nc.tensor.matmul(gm[:Rt, :Rt], sel[:, 0, :Rt], nrm[:, :Rt],
                                 start=False, stop=True)
                d2f = sbB.tile([RF, RF], F32, tag="d2f")
                nc.scalar.activation(d2f[:Rt, :Rt], gm[:Rt, :Rt], AF.Relu,
                                     scale=-2.0, bias=nf[:Rt, :])
                sf = sbB.tile([RF, RF], F32, tag="sf")
                nc.scalar.activation(sf[:Rt, :Rt], d2f[:Rt, :Rt], AF.Sqrt)
                # ---- pts dist ----
                gp = pm.tile([128, 512], F32, tag="pbig")
                nc.tensor.matmul(gp[:Rt, :Rt], nrm[0:2, :Rt], nrm[0:2, :Rt],
                                 start=True, stop=False)
                nc.tensor.matmul(gp[:Rt, :Rt], sel[:, 1, :Rt], nrm[:, :Rt],
                                 start=False, stop=True)
                d2p = sbB.tile([RF, RF], F32, tag="d2p")
                nc.scalar.activation(d2p[:Rt, :Rt], gp[:Rt, :Rt], AF.Relu,
                                     scale=-2.0, bias=np_[:Rt, :])
                sp = sbB.tile([RF, RF], F32, tag="sp")
                nc.scalar.activation(sp[:Rt, :Rt], d2p[:Rt, :Rt], AF.Sqrt)

                sfp = sbB.tile([RF, RF], F32, tag="sfp")
                nc.gpsimd.tensor_tensor(sfp[:Rt, :Rt], sf[:Rt, :Rt],
                                        sp[:Rt, :Rt], OP.add)
                score = sbB.tile([RF, RF], F32, tag="score")
                nc.vector.tensor_tensor(score[:Rt, :Rt], bo_t[:Rt, :Rt],
                                        sfp[:Rt, :Rt], OP.subtract)

                # ---- top-kv mask S ----
                mx = small.tile([RF, 8], F32, tag="mx")
                nc.vector.max(out=mx[:Rt, :], in_=score[:Rt, :Rt])
                done = small.tile([RF, 8], U32, tag="done")
                nc.vector.tensor_scalar(done[:Rt, :], aux[:Rt, 2:10], 0.0, None,
                                        OP.is_le)
                nc.vector.copy_predicated(mx[:Rt, :], done[:Rt, :], negt[:Rt, :])
                rep = sbB.tile([RF, RF], F32, tag="rep")
                nc.vector.match_replace(out=rep[:Rt, :Rt], in_to_replace=mx[:Rt, :],
                                        in_values=score[:Rt, :Rt], imm_value=NEG)
                Sb = sbA.tile([RF, RF], BF16, tag="Sb")
                nc.vector.tensor_tensor(Sb[:Rt, :Rt], score[:Rt, :Rt],
                                        rep[:Rt, :Rt], OP.is_gt)

                # ---- A matrix ----
                SRb = sbB.tile([RF, RF + 1], BF16, tag="SRb")
                nc.vector.tensor_scalar(SRb[:Rt, :Rt], Sb[:Rt, :Rt],
                                        aux[:Rt, 10:11], None, OP.mult)
                nc.vector.memset(SRb[:Rt, Rt:Rt + 1], 1.0)
                araw = pm.tile([128, 512], F32, tag="pbig")
                nc.tensor.matmul(araw[:Rt, :Rt + 1], Sb[:Rt, :Rt],
                                 SRb[:Rt, :Rt + 1], start=True, stop=True)
                dinv = small.tile([RF, 1], F32, tag="dinv")
                nc.vector.reciprocal(dinv[:Rt, :], araw[:Rt, Rt:Rt + 1])
                dz = small.tile([RF, 1], U32, tag="dz")
                nc.vector.tensor_scalar(dz[:Rt, :], araw[:Rt, Rt:Rt + 1], 0.0,
                                        None, OP.is_le)
                nc.vector.copy_predicated(dinv[:Rt, :], dz[:Rt, :], zcol[:Rt, :])
                arawb = sbA.tile([RF, RF], BF16, tag="arawb")
                nc.vector.tensor_copy(arawb[:Rt, :Rt], araw[:Rt, :Rt])
                pD = pt.tile([128, 512], F32, tag="pt")
                nc.tensor.matmul(pD[:1, :Rt], onescol[:Rt, :], Sb[:Rt, :Rt],
                                 start=True, stop=True)
                Drow = sbA.tile([1, RF], BF16, tag="Drow")
                nc.scalar.copy(Drow[:, :Rt], pD[:1, :Rt])

                # ---------------- 3 hconv layers ----------------
                h = None
                for l in range(3):
                    xt = pm.tile([128, 512], F32, tag="pbig")
                    if l == 0:
                        nc.tensor.matmul(xt[:Rt, :H], ptsTb[:, :Rt], wc0b[:],
                                         start=True, stop=True)
                    else:
                        ph = ptb.tile([128, 4, RF], BF16, tag="ptb")
                        for c in range(2):
                            nc.tensor.transpose(ph[:, c, :Rt],
                                                h[:Rt, c * 128:(c + 1) * 128],
                                                idb)
                        hTb = sbC.tile([128, 2, RF], BF16, tag="hTb")
                        nc.vector.tensor_copy(hTb[:, :, :Rt], ph[:, 0:2, :Rt])
                        nc.tensor.matmul(xt[:Rt, :H], hTb[:, 0, :Rt],
                                         wgb[:, 0, l - 1, :], start=True,
                                         stop=False)
                        nc.tensor.matmul(xt[:Rt, :H], hTb[:, 1, :Rt],
                                         wgb[:, 1, l - 1, :], start=False,
                                         stop=True)
                    xts = sbC.tile([RF, H], BF16, tag="xts")
                    if l == 1:
                        nc.vector.tensor_copy(xts[:Rt, :], xt[:Rt, :H])
                    else:
                        nc.scalar.copy(xts[:Rt, :], xt[:Rt, :H])
                    agg = pm.tile([128, 512], F32, tag="pbig")
                    nc.tensor.matmul(agg[:Rt, :H], arawb[:Rt, :Rt], xts[:Rt, :],
                                     start=True, stop=False)
                    nc.tensor.matmul(agg[:Rt, :H], Drow[:, :Rt],
                                     cmbb[:, l, :], start=False, stop=True)
                    hr = sbC.tile([RF, H], BF16, tag="hr")
                    rsum = small.tile([RF, 1], F32, tag="rsum")
                    nc.scalar.activation(hr[:Rt, :], agg[:Rt, :H], AF.Relu,
                                         scale=dinv[:Rt, :],
                                         accum_out=rsum[:Rt, :])
                    ssq = small.tile([RF, 1], F32, tag="ssq")
                    nc.scalar.activation(sqs[:Rt, :], hr[:Rt, :], AF.Square,
                                         accum_out=ssq[:Rt, :])
                    mu = small.tile([RF, 1], F32, tag="mu")
                    nc.vector.tensor_scalar_mul(mu[:Rt, :], rsum[:Rt, :], 1.0 / H)
                    mu2 = small.tile([RF, 1], F32, tag="mu2")
                    nc.vector.tensor_tensor(mu2[:Rt, :], mu[:Rt, :], mu[:Rt, :],
                                            OP.mult)
                    var = small.tile([RF, 1], F32, tag="var")
                    nc.vector.scalar_tensor_tensor(
                        var[:Rt, :], ssq[:Rt, :], 1.0 / H, mu2[:Rt, :],
                        op0=OP.mult, op1=OP.subtract)
                    sg = small.tile([RF, 1], F32, tag="sg")
                    nc.scalar.activation(sg[:Rt, :], var[:Rt, :], AF.Sqrt,
                                         bias=epsc[:Rt, :])
                    rs = small.tile([RF, 1], F32, tag="rs")
                    nc.vector.reciprocal(rs[:Rt, :], sg[:Rt, :])
                    h = sbC.tile([RF, H], BF16, tag=f"h{l}")
                    nc.vector.tensor_scalar(h[:Rt, :], hr[:Rt, :], mu[:Rt, :],
                                            rs[:Rt, :], OP.subtract, OP.mult)

                # ---------------- attention ----------------
                ph = ptb.tile([128, 4, RF], BF16, tag="ptb")
                for c in range(2):
                    nc.tensor.transpose(ph[:, c, :Rt],
                                        h[:Rt, c * 128:(c + 1) * 128], idb)
                hTb = sbC.tile([128, 2, RF], BF16, tag="hTb")
                nc.vector.tensor_copy(hTb[:, :, :Rt], ph[:, 0:2, :Rt])
                # v token-major
                pv = pm.tile([128, 512], F32, tag="pbig")
                nc.tensor.matmul(pv[:Rt, :H], hTb[:, 0, :Rt], wvb[:, 0, :],
                                 start=True, stop=False)
                nc.tensor.matmul(pv[:Rt, :H], hTb[:, 1, :Rt], wvb[:, 1, :],
                                 start=False, stop=False)
                nc.tensor.matmul(pv[:Rt, :H], ones1b[:, :Rt], cvrb[:, :],
                                 start=False, stop=True)
                vs = sbB.tile([RF, H], BF16, tag="vs")
                nc.scalar.copy(vs[:Rt, :], pv[:Rt, :H])
                # q,k feature-major: slots 0,1 = q chunks; 2,3 = k chunks
                qkT = sbC.tile([128, 4, 128], BF16, tag="qkT")
                nc.vector.memset(qkT[:, 2:4, :], 0.0)
                for o in range(4):
                    pq = pm.tile([128, 512], F32, tag="pbig")
                    nc.tensor.matmul(pq[:, :Rt], wqkb[:, 0, o, :],
                                     hTb[:, 0, :Rt], start=True, stop=False)
                    nc.tensor.matmul(pq[:, :Rt], wqkb[:, 1, o, :],
                                     hTb[:, 1, :Rt], start=False, stop=(o >= 2))
                    if o < 2:
                        nc.tensor.matmul(pq[:, :Rt], cqcb[:, o, :],
                                         ones1b[:, :Rt], start=False, stop=True)
                    if o % 2 == 0:
                        nc.vector.tensor_copy(qkT[:, o, :Rt], pq[:, :Rt])
                    else:
                        nc.scalar.copy(qkT[:, o, :Rt], pq[:, :Rt])

                # scores: one psum bank per head pair-wave + per-head mask
                scs = []
                for hh in range(4):
                    c, hf_ = hh // 2, hh % 2
                    sch = psc.tile([128, 512], F32, tag="sc")
                    nc.tensor.matmul(sch[:Rt, 0:128],
                                     qkT[64 * hf_:64 * (hf_ + 1), c, :Rt],
                                     qkT[64 * hf_:64 * (hf_ + 1), 2 + c, :],
                                     start=True, stop=False)
                    nc.tensor.matmul(sch[:Rt, 0:128], ELb[0:1 + G, :Rt],
                                     EMb[0:1 + G, hh * 128:(hh + 1) * 128],
                                     start=False, stop=True)
                    scs.append(sch)
                return dict(Rt=Rt, g0=g0, G=G, vs=vs, scs=scs)

            def tile_back(ctx, cg0, hfT):
                Rt, g0, G = ctx['Rt'], ctx['g0'], ctx['G']
                vs, scs = ctx['vs'], ctx['scs']
                idb = identb[:Rt, :Rt]
                pexp = sbB.tile([RF, 512], F32, tag="pexp")
                sums = small.tile([RF, 4], F32, tag="sums")
                for hh in range(4):
                    nc.scalar.activation(pexp[:Rt, hh * 128:(hh + 1) * 128],
                                         scs[hh][:Rt, 0:128], AF.Exp,
                                         accum_out=sums[:Rt, hh:hh + 1])
                rsum4 = small.tile([RF, 4], F32, tag="rsum4")
                nc.vector.reciprocal(rsum4[:Rt, :], sums[:Rt, :])
                att = sbB.tile([RF, 512], BF16, tag="att")
                nc.vector.tensor_tensor(
                    att[:Rt, :].rearrange("p (h j) -> p h j", h=4),
                    pexp[:Rt, :].rearrange("p (h j) -> p h j", h=4),
                    rsum4[:Rt, :, None].to_broadcast((Rt, 4, 128)),
                    OP.mult)

                # attT + AV (col-packed head pairs) -> po feature-major
                pa = ptb.tile([128, 4, RF], BF16, tag="ptb")
                for hh in range(4):
                    nc.tensor.transpose(pa[:Rt, hh, :Rt],
                                        att[:Rt, hh * 128:hh * 128 + Rt], idb)
                aT = sbB.tile([RF, 4, RF], BF16, tag="aT")
                nc.vector.tensor_copy(aT[:Rt, :, :Rt], pa[:Rt, :, :Rt])
                col0 = g0 - cg0
                for c in range(2):
                    po = pm.tile([128, 512], F32, tag="pbig")
                    for hf_ in range(2):
                        hh = 2 * c + hf_
                        nc.tensor.matmul(po[64 * hf_:64 * (hf_ + 1), :Rt],
                                         vs[:Rt, hh * 64:(hh + 1) * 64],
                                         aT[:Rt, hh, :Rt],
                                         start=True, stop=True,
                                         tile_position=(0, 64 * hf_))
                    dst = hfT[:, c, :, col0:col0 + G].rearrange("p i g -> p g i")
                    nc.vector.tensor_copy(dst, po[:, :Rt].rearrange(
                        "p (g i) -> p g i", i=J))

            def vae(cg0, cg1, hfT):
                GC = cg1 - cg0
                if GC < NV:
                    nc.vector.memset(hfT[:, :, :, GC:NV], 0.0)
                pse = [pr.tile([128, 512], F32, tag=f"pr1_{m}", name=f"pse{m}")
                       for m in range(2)]
                for kk in range(38):
                    i, half = kk // 2, kk % 2
                    for m in range(2):
                        nc.tensor.matmul(pse[m][:, :NV],
                                         we1b[:, kk, m * 128:(m + 1) * 128],
                                         hfT[:, half, i, :], start=(kk == 0),
                                         stop=(kk == 37))
                e1r = []
                for m in range(2):
                    r = sbB.tile([128, NV], BF16, tag=f"e1r{m}", name=f"e1r{m}")
                    nc.scalar.activation(r[:], pse[m][:, :NV], AF.Relu,
                                         bias=be1_t[:, m:m + 1])
                    e1r.append(r)
                # e2 -> mu, lv
                pmu = pm.tile([128, 512], F32, tag="pbig", name="pmu")
                nc.tensor.matmul(pmu[:PLAT, :NV], we2b[:, 0, 0:64], e1r[0][:],
                                 start=True, stop=False)
                nc.tensor.matmul(pmu[:PLAT, :NV], we2b[:, 1, 0:64], e1r[1][:],
                                 start=False, stop=True)
                plv = pm.tile([128, 512], F32, tag="pbig", name="plv")
                nc.tensor.matmul(plv[:PLAT, :NV], we2b[:, 0, 64:128], e1r[0][:],
                                 start=True, stop=False)
                nc.tensor.matmul(plv[:PLAT, :NV], we2b[:, 1, 64:128], e1r[1][:],
                                 start=False, stop=True)
                ex = sbB.tile([PLAT, NV], F32, tag="ex")
                nc.scalar.activation(ex[:], plv[:PLAT, :NV], AF.Exp, scale=0.5,
                                     bias=be2h_t[:, 0:1])
                exe = sbB.tile([PLAT, NV], F32, tag="exe")
                nc.vector.tensor_tensor(exe[:], ex[:], epsT_t[:, cg0:cg0 + NV],
                                        OP.mult)
                zT = sbB.tile([PLAT, NV], BF16, tag="zT")
                nc.vector.scalar_tensor_tensor(
                    zT[:], pmu[:PLAT, :NV], be2_t[:, 0:1], exe[:],
                    op0=OP.add, op1=OP.add)
                # folded dec+r1
                r1r = []
                for m in range(2):
                    prd = pm.tile([128, 512], F32, tag="pbig", name=f"prd{m}")
                    nc.tensor.matmul(prd[:, :NV], wrdb[:, m, :], zT[:],
                                     start=True, stop=True)
                    r = sbC.tile([128, NV], BF16, tag=f"r1r{m}")
                    nc.scalar.activation(r[:], prd[:, :NV], AF.Relu,
                                         bias=brd_t[:, m:m + 1])
                    r1r.append(r)
                ps = pm.tile([128, 512], F32, tag="pbig")
                nc.tensor.matmul(ps[:38, :NV], wr2b[:, 0, :], r1r[0][:],
                                 start=True, stop=False)
                nc.tensor.matmul(ps[:38, :NV], wr2b[:, 1, :], r1r[1][:],
                                 start=False, stop=True)
                predT = sbB.tile([38, NV], F32, tag="predT")
                nc.scalar.activation(predT[:], ps[:38, :NV], AF.Identity,
                                     bias=br2_t[:])
                for off in range(0, GC, 128):
                    w = min(128, GC - off)
                    p2 = pt.tile([128, 512], F32, tag="pt")
                    nc.tensor.transpose(p2[:w, :38], predT[:, off:off + w],
                                        identf[:38, :38])
                    ob = sbC.tile([128, 38], F32, tag="ob")
                    nc.vector.tensor_copy(ob[:w, :], p2[:w, :38])
                    nc.sync.dma_start(d_y[cg0 + off:cg0 + off + w, :], ob[:w, :])

            for (cg0, cg1, tl) in chunks:
                hfT = hbuf.tile([128, 2, J, NV], BF16, tag="hfT")
                for tinfo in tl:
                    ctx = process_tile(tinfo, cg0, hfT)
                    tile_back(ctx, cg0, hfT)
                vae(cg0, cg1, hfT)

    nc.finalize()
    return nc


def _host_prep(inputs, bc=BC):
    f32 = np.float32
    w_init = inputs['w_init'].astype(f32)
    b_init = inputs['b_init'].astype(f32)
    w_gnn = inputs['w_gnn'].astype(f32)
    b_gnn = inputs['b_gnn'].astype(f32)
    ln_g = inputs['ln_g'].astype(f32)
    ln_b = inputs['ln_b'].astype(f32)
    w_qkv = inputs['w_qkv'].astype(f32)
    b_qkv = inputs['b_qkv'].astype(f32)
    w_ao = inputs['w_ao'].astype(f32)
    b_ao = inputs['b_ao'].astype(f32)

    sh = {}
    wc0 = np.concatenate([w_init.T, b_init[None, :]], 0) @ w_gnn[0].T
    sh['wc0'] = wc0.astype(BF)
    wg = np.zeros((128, 2, 2, H), f32)
    cmb = np.zeros((1, 3, H), f32)
    cmb[0, 0] = b_gnn[0]
    for l in (1, 2):
        wt = (ln_g[l - 1][:, None] * w_gnn[l].T)
        wg[:, 0, l - 1, :] = wt[0:128]
        wg[:, 1, l - 1, :] = wt[128:256]
        cmb[0, l] = w_gnn[l] @ ln_b[l - 1] + b_gnn[l]
    sh['wg'] = wg.astype(BF)
    sh['cmb'] = cmb.astype(BF)
    # qkv with ln fold; q scaled 1/8; k bias dropped (softmax-invariant)
    wq = (ln_g[2][:, None] * w_qkv.T).copy()     # [256, 768]
    wq[:, 0:H] *= 0.125
    cq = w_qkv @ ln_b[2] + b_qkv
    cq[0:H] *= 0.125
    cq[H:2 * H] = 0.0
    sh['wqk'] = np.ascontiguousarray(
        wq[:, 0:512].reshape(2, 128, 4, 128).transpose(1, 0, 2, 3)).astype(BF)
    sh['wv'] = np.ascontiguousarray(
        wq[:, 512:768].reshape(2, 128, H).transpose(1, 0, 2)).astype(BF)
    sh['cqc'] = cq[0:256].reshape(1, 2, 128).astype(BF)
    sh['cvr'] = cq[512:768][None, :].astype(BF)
    # VAE: fold w_ao (and b_ao) into w_e1
    we1 = inputs['w_e1'].astype(f32)             # [256, 4864]
    we1j = we1.reshape(256, J, H)
    we1f = np.einsum('aic,cd->aid', we1j, w_ao)  # [256, J, 256]
    be1f = inputs['b_e1'].astype(f32) + np.einsum('aic,c->a', we1j, b_ao)
    sh['we1'] = np.ascontiguousarray(
        we1f.reshape(256, J * H).T.reshape(38, 128, H).transpose(1, 0, 2)
    ).astype(BF)
    sh['be1'] = np.ascontiguousarray(be1f.reshape(2, 128).T, f32)
    we2 = inputs['w_e2'].astype(f32)             # [128, 256]
    sh['we2'] = np.ascontiguousarray(
        we2.T.reshape(2, 128, 128).transpose(1, 0, 2)).astype(BF)
    sh['be2'] = np.ascontiguousarray(
        inputs['b_e2'].astype(f32).reshape(2, PLAT).T, f32)
    # fold w_r1 @ w_dec
    wdec = inputs['w_dec'].astype(f32)           # [4864, 64]
    wr1 = inputs['w_r1'].astype(f32)             # [256, 4864]
    wrd = wr1 @ wdec                             # [256, 64]
    brd = wr1 @ inputs['b_dec'].astype(f32) + inputs['b_r1'].astype(f32)
    sh['wrd'] = np.ascontiguousarray(wrd.T.reshape(64, 2, 128)).astype(BF)
    sh['brd'] = np.ascontiguousarray(brd.reshape(2, 128).T, f32)
    wr2 = inputs['w_r2'].astype(f32)             # [38, 256]
    sh['wr2'] = np.ascontiguousarray(
        wr2.T.reshape(2, 128, 38).transpose(1, 0, 2)).astype(BF)
    sh['br2'] = np.ascontiguousarray(
        inputs['b_r2'].astype(f32).reshape(38, 1), f32)
    # block-diag distance mask
    bo = np.full((RF, RF), NEG, f32)
    for g in range(GPT):
        bo[g * J:(g + 1) * J, g * J:(g + 1) * J] = 0.0
    sh['bo'] = bo
    sel = np.zeros((5, 2, RF), f32)
    sel[3, 0, :] = 1.0
    sel[4, 1, :] = 1.0
    sh['sel'] = sel
    # low-rank attention block mask rows
    EL = np.zeros((7, RF), f32)
    EM = np.zeros((7, 512), f32)
    EL[0, :] = 1.0
    EM[0, :] = -2.0e9
    for g in range(GPT):
        EL[1 + g, g * J:(g + 1) * J] = 1.0
        for hh in range(4):
            EM[1 + g, hh * 128 + g * J: hh * 128 + (g + 1) * J] = 2.0e9
    sh['EL'] = EL.astype(BF)
    sh['EM'] = EM.astype(BF)

    pts = inputs['points'].astype(f32)
    feat = inputs['img_features'].astype(f32)
    kv = inputs['k_vals']
    eps = inputs['eps'].astype(f32)
    Ba = pts.shape[0]
    aux = np.zeros((Ba * J, 11), f32)
    aux[:, 0:2] = pts.reshape(Ba * J, 2)
    aux[:, 2:10] = (kv.astype(f32).reshape(Ba * J, 1)
                    - np.arange(8, dtype=f32)[None, :])
    aux[:, 10] = 1.0 / kv.astype(f32).reshape(Ba * J)

    per_core = []
    for c in range(Ba // bc):
        g0, g1 = c * bc, (c + 1) * bc
        r0, r1 = g0 * J, g1 * J
        epsT = np.zeros((PLAT, bc + 8), f32)
        epsT[:, :bc] = eps[g0:g1].T
        m = dict(sh)
        m['feat'] = np.ascontiguousarray(feat.reshape(Ba * J, H)[r0:r1])
        m['aux'] = np.ascontiguousarray(aux[r0:r1])
        m['epsT'] = epsT
        per_core.append(m)
    return per_core


def kernel(**inputs):
    key = 'nc'
    if key not in _CACHE:
        _CACHE[key] = build_nc(BC)
    nc = _CACHE[key]
    in_maps = _host_prep(inputs, BC)
    res = run_bass_kernel_spmd(nc, in_maps, core_ids=list(range(NCORES)))
    ys = [res.results[c]['y'] for c in range(NCORES)]
    out = np.concatenate(ys, 0).reshape(B, J, 2)
    return out.astype(np.float32)
